# revision 21
# baseline (speedup 1.0000x reference)
"""Trainium2 Bass kernel for nn_MergedChunk12 (15-layer decode step, TP-8).

Sharding: every weight matmul is 8-way sharded so each weight byte is read
by exactly one core; partial sums are combined with an AllGather of
column-layout [128,F] tiles followed by a local 3-op tree fold (= AllReduce).
Attention (101 live positions at POS=100) is sharded per Q-head-half.
KV-cache passthrough (row-100 scatter) is done host-side.

Vector layout convention on device: a D-vector lives as [128, D/128] f32,
column f holding dims [128f, 128(f+1)).
"""

import numpy as np

import concourse.bass as bass
import concourse.mybir as mybir
from concourse import tile
from concourse.bass_utils import run_bass_kernel_spmd

F32 = mybir.dt.float32
F32R = mybir.dt.float32r
AF = mybir.ActivationFunctionType
AX = mybir.AxisListType

# model dims
H, NH, NKV, HD = 1024, 4, 2, 256
DFF, NL, NTOT, PLE = 2048, 15, 30, 256
W, CTX = 1024, 4096
FULL_LAYERS = (4, 9, 14)
POS = 100
EPS = 1e-6
NP_ = 100  # live cache rows (0..POS-1); row POS comes from the new k/v

N_CORES = 8
CORE_IDS = list(range(N_CORES))
RG = [CORE_IDS]


# ---------------------------------------------------------------------------
# walrus in this container rejects >1 sync-wait per instruction: hoist extras
# onto preceding NOPs on the same engine.
def _split_excess_waits(nc, max_waits=1):
    n = 0
    for fn in nc.m.functions:
        for bb in fn.blocks:
            i = 0
            insts = bb.instructions
            while i < len(insts):
                ins = insts[i]
                si = getattr(ins, 'sync_info', None)
                opc = getattr(ins, 'opcode', '') or type(ins).__name__
                limit = 0 if 'DMA' in str(opc) or 'Collective' in str(opc) else max_waits
                if si is not None and si.on_wait and len(si.on_wait) > limit:
                    waits = si.on_wait
                    keep = waits[len(waits) - limit:] if limit else []
                    extra = waits[: len(waits) - limit]
                    si.on_wait = list(keep)
                    pos = i
                    for j in range(0, len(extra), max_waits):
                        chunk = extra[j:j + max_waits]
                        nop = mybir.InstNoOp(
                            name=f"{ins.name}-wsplit-{j}",
                            engine=ins.engine,
                            ins=[], outs=[],
                            sync_info=mybir.SyncInfo(on_wait=list(chunk), on_update=[]),
                        )
                        insts.insert(pos, nop)
                        pos += 1
                        i += 1
                        n += 1
                i += 1
    return n


# ---------------------------------------------------------------------------
def build_nc():
    nc = bass.Bass()

    def inp(name, shape):
        return nc.declare_dram_parameter(name, list(shape), F32, isOutput=False)

    wqkv = inp("wqkv", (NL, 128, 2048))
    wo = inp("wo", (NL, 128, 1024))
    wgu = inp("wgu", (NL, 128, 4096))
    wd = inp("wd", (NL, 128, 2048))
    wpl = inp("wpl", (NL, 128, 256))
    plew = inp("plew", (128, 7680))
    kt = inp("kt", (NL, 128, 2 * NP_))
    vv = inp("vv", (NL, NP_, 128))
    hidc = inp("hidc", (128, 8))
    rawc = inp("rawc", (128, 60))
    plenwc = inp("plenwc", (128, 60))
    ln1c = inp("ln1c", (128, 120))
    ln2c = inp("ln2c", (128, 120))
    cosq = inp("cosq", (128, 120))
    sinq = inp("sinq", (128, 120))
    cosk = inp("cosk", (128, 60))
    sink = inp("sink", (128, 60))
    xmask = inp("xmask", (128, 8))
    qma = inp("qma", (128, 8))
    qmb = inp("qmb", (128, 8))
    qhm = inp("qhm", (128, 8))
    vmask = inp("vmask", (128, 4))
    cmask8 = inp("cmask8", (128, 8))
    onescol = inp("onescol", (128, 1))
    onesrow = inp("onesrow", (1, 128))
    onesrow16 = inp("onesrow16", (1, 128))
    ident = inp("ident", (128, 128))

    h_out = nc.declare_dram_parameter("h_out", [128, 8], F32, isOutput=True)
    ple_out = nc.declare_dram_parameter("ple_out", [128, 60], F32, isOutput=True)
    k_out = nc.declare_dram_parameter("k_out", [NL, 128, 4], F32, isOutput=True)
    v_out = nc.declare_dram_parameter("v_out", [NL, 128, 4], F32, isOutput=True)

    with nc.allow_low_precision(reason="fp32r rounding of matmul operands is intentional"), \
         tile.TileContext(nc) as tc:
        _body(
            nc, tc,
            wqkv, wo, wgu, wd, wpl, plew, kt, vv, hidc, rawc, plenwc,
            ln1c, ln2c, cosq, sinq, cosk, sink,
            xmask, qma, qmb, qhm, vmask, cmask8,
            onescol, onesrow, onesrow16, ident,
            h_out, ple_out, k_out, v_out,
        )

    _split_excess_waits(nc)
    return nc


def _body(
    nc, tc,
    wqkv, wo, wgu, wd, wpl, plew, kt, vv, hidc, rawc, plenwc,
    ln1c, ln2c, cosq, sinq, cosk, sink,
    xmask, qma, qmb, qhm, vmask, cmask8,
    onescol, onesrow, onesrow16, ident,
    h_out, ple_out, k_out, v_out,
):
    mm = nc.tensor.matmul
    dve = nc.vector
    act = nc.scalar

    def mmr(out, lhsT, rhs, **kw):
        mm(out, lhsT, rhs, **kw)

    from contextlib import ExitStack

    stack = ExitStack()
    const = stack.enter_context(tc.tile_pool(name="const", bufs=1))
    state = stack.enter_context(tc.tile_pool(name="state", bufs=2))
    wpool = stack.enter_context(tc.tile_pool(name="wpool", bufs=3))
    spool = stack.enter_context(tc.tile_pool(name="spool", bufs=2))
    dram = stack.enter_context(tc.tile_pool(name="dram", bufs=2, space="DRAM"))
    psum = stack.enter_context(tc.tile_pool(name="psum", bufs=6, space="PSUM"))

    def pstile(shape, name):
        return psum.tile(list(shape), F32, tag="ps", name=name)

    def load_const(src, shape, tag):
        t = const.tile(list(shape), F32, tag=tag, name=tag)
        nc.sync.dma_start(out=t[:], in_=src[:])
        return t

    c_plew = load_const(plew, (128, 7680), "c_plew")
    c_hid = load_const(hidc, (128, 8), "c_hid")
    c_raw = load_const(rawc, (128, 60), "c_raw")
    c_plenw = load_const(plenwc, (128, 60), "c_plenw")
    c_ln1 = load_const(ln1c, (128, 120), "c_ln1")
    c_ln2 = load_const(ln2c, (128, 120), "c_ln2")
    c_cosq = load_const(cosq, (128, 120), "c_cosq")
    c_sinq = load_const(sinq, (128, 120), "c_sinq")
    c_cosk = load_const(cosk, (128, 60), "c_cosk")
    c_sink = load_const(sink, (128, 60), "c_sink")
    c_xm = load_const(xmask, (128, 8), "c_xm")
    c_qma = load_const(qma, (128, 8), "c_qma")
    c_qmb = load_const(qmb, (128, 8), "c_qmb")
    c_qhm = load_const(qhm, (128, 8), "c_qhm")
    c_vm = load_const(vmask, (128, 4), "c_vm")
    c_cm8 = load_const(cmask8, (128, 8), "c_cm8")
    c_one = load_const(onescol, (128, 1), "c_one")
    c_oner = load_const(onesrow, (1, 128), "c_oner")
    c_oner16 = load_const(onesrow16, (1, 128), "c_oner16")
    c_id = load_const(ident, (128, 128), "c_id")

    # ---- cross-core sum of a column-layout [128, F] SBUF tile ----
    ar_idx = [0]

    def allreduce(send_ap, F, tag):
        i = ar_idx[0]
        ar_idx[0] += 1
        inb = dram.tile([128, F], F32, tag=f"{tag}_in", name=f"{tag}_in{i}")
        outb = dram.tile(
            [8 * 128, F], F32, tag=f"{tag}_out", name=f"{tag}_out{i}",
            addr_space="Shared",
        )
        nc.scalar.dma_start(out=inb[:], in_=send_ap)
        # PE-warming dummies: keep the tensor engine busy through the
        # collective's latency so real matmuls run at the warm clock.
        for wdi in range(6):
            wps = psum.tile([1, 512], F32, tag="warm", name=f"warm{i}_{wdi}", bufs=2)
            mm(wps[:], send_ap[:, 0:1], c_plew[:, 512 * wdi : 512 * (wdi + 1)],
               start=True, stop=True)
        nc.gpsimd.collective_compute(
            "AllGather",
            mybir.AluOpType.bypass,
            replica_groups=RG,
            ins=[inb.opt()],
            outs=[outb.opt()],
        )
        slots = spool.tile([128, 8 * F], F32, tag=f"{tag}_sl", name=f"{tag}_sl{i}")
        nc.scalar.dma_start(
            out=slots.rearrange("p (s f) -> p s f", s=8),
            in_=outb.rearrange("(s p) f -> p s f", p=128),
        )
        a = spool.tile([128, 4 * F], F32, tag=f"{tag}_a", name=f"{tag}_a{i}")
        dve.tensor_add(a[:], slots[:, : 4 * F], slots[:, 4 * F :])
        b = spool.tile([128, 2 * F], F32, tag=f"{tag}_b", name=f"{tag}_b{i}")
        dve.tensor_add(b[:], a[:, : 2 * F], a[:, 2 * F :])
        c = spool.tile([128, F], F32, tag=f"{tag}_c", name=f"{tag}_c{i}")
        dve.tensor_add(c[:], b[:, :F], b[:, F:])
        return c

    # ---- broadcast a [1, n] row of per-column scalars to [128, n] (SBUF) ----
    bc_idx = [0]

    def bcast_cols(row_ap, n, tag, ones=None):
        i = bc_idx[0]
        bc_idx[0] += 1
        t = pstile([128, n], f"bc_{tag}{i}")
        mm(t[:], (ones if ones is not None else c_oner)[:], row_ap,
           start=True, stop=True)
        s = spool.tile([128, n], F32, tag=f"bc_{tag}", name=f"bcs_{tag}{i}")
        dve.tensor_copy(s[:], t[:])
        return s

    # ---- rms scale for a [128, F] column-layout vector, D = 128*F ----
    rm_idx = [0]

    def global_rms_scale(v, F, D, tag):
        i = rm_idx[0]
        rm_idx[0] += 1
        sq = spool.tile([128, F], F32, tag=f"rm_sq_{tag}", name=f"rm_sq_{tag}{i}")
        dve.tensor_mul(sq[:], v[:], v[:])
        cs = pstile([1, F], f"rm_cs_{tag}{i}")
        mm(cs[:], c_one[:], sq[:], start=True, stop=True)
        tot = spool.tile([1, 1], F32, tag=f"rm_t_{tag}", name=f"rm_t_{tag}{i}")
        dve.reduce_sum(tot[:], cs[:], axis=AX.X)
        tt = spool.tile([1, 1], F32, tag=f"rm_tt_{tag}", name=f"rm_tt_{tag}{i}")
        dve.tensor_scalar(tt[:], tot[:], 1.0 / D, EPS,
                          op0=mybir.AluOpType.mult, op1=mybir.AluOpType.add)
        sr = spool.tile([1, 1], F32, tag=f"rm_s_{tag}", name=f"rm_s_{tag}{i}")
        act.activation(sr[:], tt[:], AF.Sqrt)
        rs = spool.tile([1, 1], F32, tag=f"rm_r_{tag}", name=f"rm_r_{tag}{i}")
        dve.reciprocal(rs[:], sr[:])
        scp = pstile([128, 1], f"rm_b_{tag}{i}")
        mm(scp[:], c_oner[:], rs[:], start=True, stop=True)
        scs = spool.tile([128, 1], F32, tag=f"rm_bs_{tag}", name=f"rm_bs_{tag}{i}")
        dve.tensor_copy(scs[:], scp[:])
        return scs

    # =========================== PLE stage ===============================
    xple = state.tile([128, 1], F32, tag="xple", name="xple")
    tmp8 = state.tile([128, 8], F32, tag="tmp8", name="tmp8")
    dve.tensor_mul(tmp8[:], c_hid[:], c_xm[:])
    dve.reduce_sum(xple[:], tmp8[:], axis=AX.X)

    pj = pstile([128, 60], "pj")
    for m in range(60):
        mmr(pj[:, m : m + 1], c_plew[:, 128 * m : 128 * (m + 1)], xple[:],
            start=True, stop=True)
    pjs = state.tile([128, 60], F32, tag="pjs", name="pjs")
    dve.tensor_copy(pjs[:], pj[:])
    projs = allreduce(pjs[:], 60, "arp")

    # group-rms over 30 groups of 256 (cols 2g, 2g+1)
    psq = state.tile([128, 60], F32, tag="psq", name="psq")
    dve.tensor_mul(psq[:], projs[:], projs[:])
    pcs = pstile([1, 60], "pcs")
    mm(pcs[:], c_one[:], psq[:], start=True, stop=True)
    pcss = state.tile([1, 60], F32, tag="pcss", name="pcss")
    dve.tensor_copy(pcss[:], pcs[:])
    g2 = state.tile([1, 30], F32, tag="g2", name="g2")
    dve.tensor_add(g2[:], pcss[:, 0::2], pcss[:, 1::2])
    g2e = state.tile([1, 30], F32, tag="g2e", name="g2e")
    dve.tensor_scalar(g2e[:], g2[:], 1.0 / PLE, EPS,
                      op0=mybir.AluOpType.mult, op1=mybir.AluOpType.add)
    gsr = state.tile([1, 30], F32, tag="gsr", name="gsr")
    act.activation(gsr[:], g2e[:], AF.Sqrt)
    grs = state.tile([1, 30], F32, tag="grs", name="grs")
    dve.reciprocal(grs[:], gsr[:])
    s60 = state.tile([1, 60], F32, tag="s60", name="s60")
    dve.tensor_copy(s60[:, 0::2], grs[:])
    dve.tensor_copy(s60[:, 1::2], grs[:])
    sc60 = bcast_cols(s60[:], 60, "p60")
    normed = state.tile([128, 60], F32, tag="normed", name="normed")
    dve.tensor_mul(normed[:], projs[:], sc60[:])
    # plenw and raw are pre-scaled by 2^-0.5 on host
    plec = state.tile([128, 60], F32, tag="plec", name="plec")
    dve.tensor_mul(plec[:], normed[:], c_plenw[:])
    dve.tensor_add(plec[:], plec[:], c_raw[:])
    nc.sync.dma_start(out=ple_out[:], in_=plec[:])

    # =========================== layers ==================================
    h = state.tile([128, 8], F32, tag="h", name="h0")
    dve.tensor_copy(h[:], c_hid[:])

    for l in range(NL):
        w_qkv = wpool.tile([128, 2048], F32, tag="w_qkv", name=f"w_qkv{l}")
        nc.sync.dma_start(out=w_qkv[:], in_=wqkv[l])
        w_o = wpool.tile([128, 1024], F32, tag="w_o", name=f"w_o{l}")
        nc.sync.dma_start(out=w_o[:], in_=wo[l])
        w_gu = wpool.tile([128, 4096], F32, tag="w_gu", name=f"w_gu{l}")
        nc.sync.dma_start(out=w_gu[:], in_=wgu[l])
        w_d = wpool.tile([128, 2048], F32, tag="w_d", name=f"w_d{l}")
        nc.sync.dma_start(out=w_d[:], in_=wd[l])
        w_pl = wpool.tile([128, 256], F32, tag="w_pl", name=f"w_pl{l}")
        nc.sync.dma_start(out=w_pl[:], in_=wpl[l])
        t_kt = wpool.tile([128, 2 * NP_], F32, tag="t_kt", name=f"t_kt{l}")
        nc.sync.dma_start(out=t_kt[:], in_=kt[l])
        vsb = wpool.tile([NP_, 128], F32, tag="vsb", name=f"vsb{l}")
        nc.sync.dma_start(out=vsb[:], in_=vv[l])

        # ---- x = rms(h) * ln1 ----
        hsc = global_rms_scale(h, 8, H, "h1")
        x = spool.tile([128, 8], F32, tag="x", name=f"x{l}")
        dve.tensor_scalar_mul(x[:], h[:], hsc[:])
        dve.tensor_mul(x[:], x[:], c_ln1[:, 8 * l : 8 * l + 8])

        # ---- qkv partials ----
        xc8 = spool.tile([128, 8], F32, tag="xc8", name=f"xc8{l}")
        dve.tensor_mul(xc8[:], x[:], c_xm[:])
        xcs = spool.tile([128, 1], F32, tag="xcs", name=f"xcs{l}")
        dve.reduce_sum(xcs[:], xc8[:], axis=AX.X)
        pqkv = pstile([128, 16], f"pqkv{l}")
        for m in range(16):
            mmr(pqkv[:, m : m + 1], w_qkv[:, 128 * m : 128 * (m + 1)], xcs[:],
                start=True, stop=True)
        sqkv = spool.tile([128, 16], F32, tag="sqkv", name=f"sqkv{l}")
        dve.tensor_copy(sqkv[:], pqkv[:])
        qkvc = allreduce(sqkv[:], 16, "ar1")
        q = qkvc[:, 0:8]
        k = qkvc[:, 8:12]
        v = qkvc[:, 12:16]

        # v straight to cache output
        nc.sync.dma_start(out=v_out[l], in_=v)

        # ---- q rms (with 1/16 folded) + rope ----
        qsq = spool.tile([128, 8], F32, tag="qsq", name=f"qsq{l}")
        dve.tensor_mul(qsq[:], q, q)
        qcs = pstile([1, 8], f"qcs{l}")
        mm(qcs[:], c_one[:], qsq[:], start=True, stop=True)
        qcss = spool.tile([1, 8], F32, tag="qcss", name=f"qcss{l}")
        dve.tensor_copy(qcss[:], qcs[:])
        qg = spool.tile([1, 4], F32, tag="qg", name=f"qg{l}")
        dve.tensor_add(qg[:], qcss[:, 0::2], qcss[:, 1::2])
        qge = spool.tile([1, 4], F32, tag="qge", name=f"qge{l}")
        dve.tensor_scalar(qge[:], qg[:], 256.0 / HD, 256.0 * EPS,
                          op0=mybir.AluOpType.mult, op1=mybir.AluOpType.add)
        qsr = spool.tile([1, 4], F32, tag="qsr", name=f"qsr{l}")
        act.activation(qsr[:], qge[:], AF.Sqrt)
        qrs = spool.tile([1, 4], F32, tag="qrs", name=f"qrs{l}")
        dve.reciprocal(qrs[:], qsr[:])
        qs8 = spool.tile([1, 8], F32, tag="qs8", name=f"qs8{l}")
        dve.tensor_copy(qs8[:, 0::2], qrs[:])
        dve.tensor_copy(qs8[:, 1::2], qrs[:])
        qsc = bcast_cols(qs8[:], 8, "q")  # rsqrt/16 folded via 256x sqrt arg
        qn_ = spool.tile([128, 8], F32, tag="qn_", name=f"qn_{l}")
        dve.tensor_mul(qn_[:], q, qsc[:])
        rotq = spool.tile([128, 8], F32, tag="rotq", name=f"rotq{l}")
        dve.tensor_scalar_mul(rotq[:, 0::2], qn_[:, 1::2], -1.0)
        dve.tensor_copy(rotq[:, 1::2], qn_[:, 0::2])
        qr = spool.tile([128, 8], F32, tag="qr", name=f"qr{l}")
        dve.tensor_mul(qr[:], qn_[:], c_cosq[:, 8 * l : 8 * l + 8])
        dve.tensor_mul(rotq[:], rotq[:], c_sinq[:, 8 * l : 8 * l + 8])
        dve.tensor_add(qr[:], qr[:], rotq[:])

        # ---- k rms + rope ----
        ksq = spool.tile([128, 4], F32, tag="ksq", name=f"ksq{l}")
        dve.tensor_mul(ksq[:], k, k)
        kcs = pstile([1, 4], f"kcs{l}")
        mm(kcs[:], c_one[:], ksq[:], start=True, stop=True)
        kcss = spool.tile([1, 4], F32, tag="kcss", name=f"kcss{l}")
        dve.tensor_copy(kcss[:], kcs[:])
        kg = spool.tile([1, 2], F32, tag="kg", name=f"kg{l}")
        dve.tensor_add(kg[:], kcss[:, 0::2], kcss[:, 1::2])
        kge = spool.tile([1, 2], F32, tag="kge", name=f"kge{l}")
        dve.tensor_scalar(kge[:], kg[:], 1.0 / HD, EPS,
                          op0=mybir.AluOpType.mult, op1=mybir.AluOpType.add)
        ksr = spool.tile([1, 2], F32, tag="ksr", name=f"ksr{l}")
        act.activation(ksr[:], kge[:], AF.Sqrt)
        krs = spool.tile([1, 2], F32, tag="krs", name=f"krs{l}")
        dve.reciprocal(krs[:], ksr[:])
        ks4 = spool.tile([1, 4], F32, tag="ks4", name=f"ks4{l}")
        dve.tensor_copy(ks4[:, 0::2], krs[:])
        dve.tensor_copy(ks4[:, 1::2], krs[:])
        ksc = bcast_cols(ks4[:], 4, "k")
        dwe = spool.tile([1, 1], F32, tag="dwe", name=f"dwe{l}")
        act.activation(dwe[:], krs[:, 0:1], AF.Exp)  # pre-warm Exp table
        kn_ = spool.tile([128, 4], F32, tag="kn_", name=f"kn_{l}")
        dve.tensor_mul(kn_[:], k, ksc[:])
        rotk = spool.tile([128, 4], F32, tag="rotk", name=f"rotk{l}")
        dve.tensor_scalar_mul(rotk[:, 0::2], kn_[:, 1::2], -1.0)
        dve.tensor_copy(rotk[:, 1::2], kn_[:, 0::2])
        kr = spool.tile([128, 4], F32, tag="kr", name=f"kr{l}")
        dve.tensor_mul(kr[:], kn_[:], c_cosk[:, 4 * l : 4 * l + 4])
        dve.tensor_mul(rotk[:], rotk[:], c_sink[:, 4 * l : 4 * l + 4])
        dve.tensor_add(kr[:], kr[:], rotk[:])
        nc.sync.dma_start(out=k_out[l], in_=kr[:])

        # ---- my q head halves ----
        tq = spool.tile([128, 8], F32, tag="tq", name=f"tq{l}")
        dve.tensor_mul(tq[:], qr[:], c_qma[:])
        qa = spool.tile([128, 1], F32, tag="qa", name=f"qa{l}")
        dve.reduce_sum(qa[:], tq[:], axis=AX.X)
        dve.tensor_mul(tq[:], qr[:], c_qmb[:])
        qb = spool.tile([128, 1], F32, tag="qb", name=f"qb{l}")
        dve.reduce_sum(qb[:], tq[:], axis=AX.X)

        # ---- scores over cache rows 0..99 ----
        psc = pstile([1, POS + 1], f"psc{l}")
        mm(psc[:, 0:NP_], qa[:], t_kt[:, 0:NP_], start=True, stop=False)
        mm(psc[:, 0:NP_], qb[:], t_kt[:, NP_ : 2 * NP_], start=False, stop=True)

        # ---- score at row 100 (new k) ----
        krep = spool.tile([128, 8], F32, tag="krep", name=f"krep{l}")
        dve.tensor_copy(krep[:, 0:2], kr[:, 0:2])
        dve.tensor_copy(krep[:, 2:4], kr[:, 0:2])
        dve.tensor_copy(krep[:, 4:6], kr[:, 2:4])
        dve.tensor_copy(krep[:, 6:8], kr[:, 2:4])
        dve.tensor_mul(krep[:], krep[:], qr[:])
        dve.tensor_mul(krep[:], krep[:], c_qhm[:])
        s1cs = pstile([1, 8], f"s1cs{l}")
        mm(s1cs[:], c_one[:], krep[:], start=True, stop=True)
        s100 = spool.tile([1, 1], F32, tag="s100", name=f"s100{l}")
        dve.reduce_sum(s100[:], s1cs[:], axis=AX.X)

        sc = spool.tile([1, POS + 1], F32, tag="sc", name=f"sc{l}")
        dve.tensor_copy(sc[:, 0:NP_], psc[:, 0:NP_])
        dve.tensor_copy(sc[:, POS : POS + 1], s100[:])

        # ---- softmax ----
        mx = spool.tile([1, 1], F32, tag="mx", name=f"mx{l}")
        dve.reduce_max(mx[:], sc[:], axis=AX.X)
        nmx = spool.tile([1, 1], F32, tag="nmx", name=f"nmx{l}")
        dve.tensor_scalar_mul(nmx[:], mx[:], -1.0)
        e = spool.tile([1, POS + 1], F32, tag="e", name=f"e{l}")
        esum = spool.tile([1, 1], F32, tag="esum", name=f"esum{l}")
        act.activation(e[:], sc[:], AF.Exp, bias=nmx[:], scale=1.0,
                       accum_out=esum[:])
        dws = spool.tile([1, 1], F32, tag="dws", name=f"dws{l}")
        act.activation(dws[:], esum[:], AF.Sqrt)  # pre-warm Sqrt table
        rcp = spool.tile([1, 1], F32, tag="rcp", name=f"rcp{l}")
        dve.reciprocal(rcp[:], esum[:])
        p = spool.tile([1, POS + 1], F32, tag="p", name=f"p{l}")
        dve.tensor_scalar_mul(p[:], e[:], rcp[:])

        # ---- p[:100] to column; v_new row ----
        pct = pstile([NP_, 1], f"pct{l}")
        nc.tensor.transpose(pct[:], p[:, 0:NP_], c_id[0:1, 0:1])
        pcol = spool.tile([NP_, 1], F32, tag="pcol", name=f"pcol{l}")
        dve.tensor_copy(pcol[:], pct[:])

        tv8 = spool.tile([128, 4], F32, tag="tv8", name=f"tv8{l}")
        dve.tensor_mul(tv8[:], v, c_vm[:])
        vsl = spool.tile([128, 1], F32, tag="vsl", name=f"vsl{l}")
        dve.reduce_sum(vsl[:], tv8[:], axis=AX.X)
        vrt = pstile([1, 128], f"vrt{l}")
        nc.tensor.transpose(vrt[:], vsl[:], c_id[:])
        vrow = spool.tile([1, 128], F32, tag="vrow", name=f"vrow{l}")
        dve.tensor_copy(vrow[:], vrt[:])

        # ---- att = V[0:100]^T p[0:100] + p[100] * v_new ----
        patt = pstile([128, 1], f"patt{l}")
        mm(patt[:], vsb[:], pcol[:], start=True, stop=False)
        mm(patt[:], vrow[:], p[:, POS : POS + 1], start=False, stop=True)
        attc = spool.tile([128, 1], F32, tag="attc", name=f"attc{l}")
        dve.tensor_copy(attc[:], patt[:])

        # ---- o partials ----
        po = pstile([128, 8], f"po{l}")
        for m in range(8):
            mmr(po[:, m : m + 1], w_o[:, 128 * m : 128 * (m + 1)], attc[:],
                start=True, stop=True)
        so = spool.tile([128, 8], F32, tag="so", name=f"so{l}")
        dve.tensor_copy(so[:], po[:])
        oc = allreduce(so[:], 8, "ar2")
        h2 = state.tile([128, 8], F32, tag="h2", name=f"h2_{l}")
        dve.tensor_add(h2[:], h[:], oc[:])

        # ---- x2 = rms(h2) * ln2 ----
        h2sc = global_rms_scale(h2, 8, H, "h2")
        dwg = spool.tile([1, 1], F32, tag="dwg", name=f"dwg{l}")
        act.activation(dwg[:], h2sc[0:1, :], AF.Gelu_apprx_tanh)  # pre-warm
        x2 = spool.tile([128, 8], F32, tag="x2", name=f"x2{l}")
        dve.tensor_scalar_mul(x2[:], h2[:], h2sc[:])
        dve.tensor_mul(x2[:], x2[:], c_ln2[:, 8 * l : 8 * l + 8])

        # ---- mlp g/u (col shard: full x2 contraction) ----
        pg = pstile([128, 2], f"pg{l}")
        pu = pstile([128, 2], f"pu{l}")
        for m in range(2):
            for kk in range(8):
                mmr(pg[:, m : m + 1],
                    w_gu[:, 256 * kk + 128 * m : 256 * kk + 128 * m + 128],
                    x2[:, kk : kk + 1], start=(kk == 0), stop=(kk == 7))
        for m in range(2):
            for kk in range(8):
                mmr(pu[:, m : m + 1],
                    w_gu[:, 2048 + 256 * kk + 128 * m : 2048 + 256 * kk + 128 * m + 128],
                    x2[:, kk : kk + 1], start=(kk == 0), stop=(kk == 7))
        ga = spool.tile([128, 2], F32, tag="ga", name=f"ga{l}")
        act.activation(ga[:], pg[:], AF.Gelu_apprx_tanh)
        mc = spool.tile([128, 2], F32, tag="mc", name=f"mc{l}")
        dve.tensor_mul(mc[:], ga[:], pu[:])
        dwq = spool.tile([1, 1], F32, tag="dwq", name=f"dwq{l}")
        act.activation(dwq[:], ga[0:1, 0:1], AF.Sqrt)  # pre-warm Sqrt table

        # ---- d partials (row shard of Wd over my 256 dff dims) ----
        pd = pstile([128, 8], f"pd{l}")
        for m in range(8):
            for kk in range(2):
                mmr(pd[:, m : m + 1],
                    w_d[:, 1024 * kk + 128 * m : 1024 * kk + 128 * m + 128],
                    mc[:, kk : kk + 1], start=(kk == 0), stop=(kk == 1))
        # ---- ple contribution (col shard of Wpl; placed by col mask) ----
        ppl = pstile([128, 1], f"ppl{l}")
        for kk in range(2):
            mmr(ppl[:], w_pl[:, 128 * kk : 128 * kk + 128],
                plec[:, 2 * l + kk : 2 * l + kk + 1],
                start=(kk == 0), stop=(kk == 1))
        plp = spool.tile([128, 1], F32, tag="plp", name=f"plp{l}")
        dve.tensor_copy(plp[:], ppl[:])
        sd = spool.tile([128, 8], F32, tag="sd", name=f"sd{l}")
        dve.tensor_scalar_mul(sd[:], c_cm8[:], plp[:])
        dve.tensor_add(sd[:], sd[:], pd[:])

        dc = allreduce(sd[:], 8, "ar3")
        hn = state.tile([128, 8], F32, tag="h", name=f"h{l + 1}")
        dve.tensor_add(hn[:], h2[:], dc[:])
        h = hn

    nc.sync.dma_start(out=h_out[:], in_=h[:])

    stack.close()


# ---------------------------------------------------------------------------
# host-side shard prep
def _col(vec, parts=128):
    """[D] -> [128, D/128] column layout (col f = dims 128f..128f+128)."""
    v = np.asarray(vec, dtype=np.float32).reshape(-1)
    return np.ascontiguousarray(v.reshape(-1, parts).T)


def _uncol(mat):
    return np.ascontiguousarray(mat.T).reshape(-1)


def _host_prep(inp):
    f32 = np.float32
    Wq = np.asarray(inp["Wq"], f32)
    Wk = np.asarray(inp["Wk"], f32)
    Wv = np.asarray(inp["Wv"], f32)
    Wo = np.asarray(inp["Wo"], f32)
    Wg = np.asarray(inp["Wg"], f32)
    Wu = np.asarray(inp["Wu"], f32)
    Wd = np.asarray(inp["Wd"], f32)
    Wpl = np.asarray(inp["Wpl"], f32)
    plw = np.asarray(inp["ple_proj_W"], f32)
    qn = np.asarray(inp["qn"], f32)
    kn = np.asarray(inp["kn"], f32)
    ln1 = np.asarray(inp["ln1"], f32)
    ln2 = np.asarray(inp["ln2"], f32)
    Ks = np.asarray(inp["K_sliding_in"], f32)
    Vs = np.asarray(inp["V_sliding_in"], f32)
    Kf = np.asarray(inp["K_full_in"], f32)
    Vf = np.asarray(inp["V_full_in"], f32)
    cos_s = np.asarray(inp["cos_s"], f32).reshape(HD)
    sin_s = np.asarray(inp["sin_s"], f32).reshape(HD)
    cos_f = np.asarray(inp["cos_f"], f32).reshape(HD)
    sin_f = np.asarray(inp["sin_f"], f32).reshape(HD)
    hid = np.asarray(inp["hidden_states"], f32).reshape(H)
    raw = np.asarray(inp["per_layer_raw"], f32).reshape(NTOT * PLE)
    plnw = np.asarray(inp["ple_norm_w"], f32).reshape(PLE)

    # per-layer cache K/V (sliding/full), rows 0..99 of the right kv slice
    kcache = []
    vcache = []
    si = fi = 0
    for l in range(NL):
        if l in FULL_LAYERS:
            kcache.append(Kf[fi, 0])
            vcache.append(Vf[fi, 0])
            fi += 1
        else:
            kcache.append(Ks[si, 0])
            vcache.append(Vs[si, 0])
            si += 1

    # per-layer cos/sin columns with qn/kn folded
    cosq_h = np.zeros((128, 120), f32)
    sinq_h = np.zeros((128, 120), f32)
    cosk_h = np.zeros((128, 60), f32)
    sink_h = np.zeros((128, 60), f32)
    ln1_h = np.zeros((128, 120), f32)
    ln2_h = np.zeros((128, 120), f32)
    for l in range(NL):
        cos = cos_f if l in FULL_LAYERS else cos_s
        sin = sin_f if l in FULL_LAYERS else sin_s
        qnl, knl = qn[l], kn[l]
        qn_sw = np.concatenate([qnl[128:], qnl[:128]])
        kn_sw = np.concatenate([knl[128:], knl[:128]])
        cq = _col(cos * qnl)      # [128,2]
        sq_ = _col(sin * qn_sw)
        ck = _col(cos * knl)
        sk = _col(sin * kn_sw)
        for u in range(NH):
            cosq_h[:, 8 * l + 2 * u : 8 * l + 2 * u + 2] = cq
            sinq_h[:, 8 * l + 2 * u : 8 * l + 2 * u + 2] = sq_
        for u in range(NKV):
            cosk_h[:, 4 * l + 2 * u : 4 * l + 2 * u + 2] = ck
            sink_h[:, 4 * l + 2 * u : 4 * l + 2 * u + 2] = sk
        ln1_h[:, 8 * l : 8 * l + 8] = _col(ln1[l])
        ln2_h[:, 8 * l : 8 * l + 8] = _col(ln2[l])

    common = dict(
        hidc=_col(hid),
        rawc=_col(raw) * np.float32(2 ** -0.5),
        plenwc=np.tile(_col(plnw), (1, 30)).astype(f32) * np.float32(2 ** -0.5),
        ln1c=ln1_h, ln2c=ln2_h,
        cosq=cosq_h, sinq=sinq_h, cosk=cosk_h, sink=sink_h,
        onescol=np.ones((128, 1), f32),
        onesrow=np.ones((1, 128), f32),
        onesrow16=np.full((1, 128), HD ** -0.5, f32),
        ident=np.eye(128, dtype=f32),
    )

    in_maps = []
    for c in range(N_CORES):
        r0 = 128 * c
        j = c // 2          # q head
        half = c % 2
        jk = c // 4         # kv head
        wqkv_c = np.concatenate(
            [Wq[:, r0 : r0 + 128, :], Wk[:, r0 : r0 + 128, :],
             Wv[:, r0 : r0 + 128, :]], axis=2)
        worow = 256 * j + 128 * half
        wo_c = Wo[:, worow : worow + 128, :]
        gcols = slice(256 * c, 256 * c + 256)
        wg_c = (Wg[:, :, gcols].reshape(NL, 8, 128, 256)
                .transpose(0, 2, 1, 3).reshape(NL, 128, 2048))
        wu_c = (Wu[:, :, gcols].reshape(NL, 8, 128, 256)
                .transpose(0, 2, 1, 3).reshape(NL, 128, 2048))
        wgu_c = np.concatenate([wg_c, wu_c], axis=2)
        wd_c = (Wd[:, 256 * c : 256 * c + 256, :].reshape(NL, 2, 128, 1024)
                .transpose(0, 2, 1, 3).reshape(NL, 128, 2048))
        wpl_c = (Wpl[:, :, 128 * c : 128 * c + 128].reshape(NL, 2, 128, 128)
                 .transpose(0, 2, 1, 3).reshape(NL, 128, 256))
        plew_c = plw[r0 : r0 + 128, :] * np.float32(H ** -0.5)

        kt_c = np.zeros((NL, 128, 2 * NP_), f32)
        vv_c = np.zeros((NL, NP_, 128), f32)
        for l in range(NL):
            kk = kcache[l][0:NP_, 256 * jk : 256 * jk + 256]  # [100,256]
            ktt = np.ascontiguousarray(kk.T).reshape(2, 128, NP_)
            kt_c[l, :, 0:NP_] = ktt[0]
            kt_c[l, :, NP_ : 2 * NP_] = ktt[1]
            vcol0 = 256 * jk + 128 * half
            vv_c[l] = vcache[l][0:NP_, vcol0 : vcol0 + 128]

        def onehot(ncols, hot):
            m = np.zeros((128, ncols), f32)
            for hh in (hot if isinstance(hot, (list, tuple)) else [hot]):
                m[:, hh] = 1.0
            return m

        im = dict(common)
        im.update(
            wqkv=np.ascontiguousarray(wqkv_c),
            wo=np.ascontiguousarray(wo_c),
            wgu=np.ascontiguousarray(wgu_c),
            wd=np.ascontiguousarray(wd_c),
            wpl=np.ascontiguousarray(wpl_c),
            plew=np.ascontiguousarray(plew_c),
            kt=kt_c,
            vv=vv_c,
            xmask=onehot(8, c),
            qma=onehot(8, 2 * j),
            qmb=onehot(8, 2 * j + 1),
            qhm=onehot(8, [2 * j, 2 * j + 1]),
            vmask=onehot(4, 2 * jk + half),
            cmask8=onehot(8, c),
        )
        in_maps.append(im)
    return in_maps


_CACHED_NC = None


def _get_nc():
    global _CACHED_NC
    if _CACHED_NC is None:
        _CACHED_NC = build_nc()
    return _CACHED_NC


def kernel(_trace=False, **inputs):
    nc = _get_nc()
    in_maps = _host_prep(inputs)
    res = run_bass_kernel_spmd(nc, in_maps, CORE_IDS, trace=_trace)
    r0 = res.results[0]

    h = _uncol(r0["h_out"]).reshape(1, 1, H)
    ple = _uncol(r0["ple_out"]).reshape(1, 1, NTOT * PLE)
    k_new = np.stack([_uncol(r0["k_out"][l]) for l in range(NL)])  # [15,512]
    v_new = np.stack([_uncol(r0["v_out"][l]) for l in range(NL)])

    Ks = np.array(np.asarray(inputs["K_sliding_in"], np.float32), copy=True)
    Vs = np.array(np.asarray(inputs["V_sliding_in"], np.float32), copy=True)
    Kf = np.array(np.asarray(inputs["K_full_in"], np.float32), copy=True)
    Vf = np.array(np.asarray(inputs["V_full_in"], np.float32), copy=True)
    si = fi = 0
    kv13 = kv14 = None
    for l in range(NL):
        if l in FULL_LAYERS:
            Kf[fi, 0, POS, :] = k_new[l]
            Vf[fi, 0, POS, :] = v_new[l]
            fi += 1
        else:
            Ks[si, 0, POS, :] = k_new[l]
            Vs[si, 0, POS, :] = v_new[l]
            si += 1
    kv13_k = k_new[13][:HD].reshape(1, 1, 1, HD)
    kv13_v = v_new[13][:HD].reshape(1, 1, 1, HD)
    kv14_k = k_new[14].reshape(1, 1, 1, NKV * HD)
    kv14_v = v_new[14].reshape(1, 1, 1, NKV * HD)

    out = (h, Ks, Vs, Kf, Vf, kv13_k, kv13_v, kv14_k, kv14_v, ple)
    if _trace:
        return out, res
    return out


# revision 22
# speedup vs baseline: 1.0306x; 1.0306x over previous
"""Trainium2 Bass kernel for nn_MergedChunk12 (15-layer decode step, TP-8).

Sharding: every weight matmul is 8-way sharded so each weight byte is read
by exactly one core; partial sums are combined with an AllGather of
column-layout [128,F] tiles followed by a local 3-op tree fold (= AllReduce).
Attention (101 live positions at POS=100) is sharded per Q-head-half.
KV-cache passthrough (row-100 scatter) is done host-side.

Vector layout convention on device: a D-vector lives as [128, D/128] f32,
column f holding dims [128f, 128(f+1)).
"""

import numpy as np

import concourse.bass as bass
import concourse.mybir as mybir
from concourse import tile
from concourse.bass_utils import run_bass_kernel_spmd

F32 = mybir.dt.float32
F32R = mybir.dt.float32r
AF = mybir.ActivationFunctionType
AX = mybir.AxisListType

# model dims
H, NH, NKV, HD = 1024, 4, 2, 256
DFF, NL, NTOT, PLE = 2048, 15, 30, 256
W, CTX = 1024, 4096
FULL_LAYERS = (4, 9, 14)
POS = 100
EPS = 1e-6
NP_ = 100  # live cache rows (0..POS-1); row POS comes from the new k/v

N_CORES = 8
CORE_IDS = list(range(N_CORES))
RG = [CORE_IDS]


# ---------------------------------------------------------------------------
# walrus in this container rejects >1 sync-wait per instruction: hoist extras
# onto preceding NOPs on the same engine.
def _split_excess_waits(nc, max_waits=1):
    n = 0
    for fn in nc.m.functions:
        for bb in fn.blocks:
            i = 0
            insts = bb.instructions
            while i < len(insts):
                ins = insts[i]
                si = getattr(ins, 'sync_info', None)
                opc = getattr(ins, 'opcode', '') or type(ins).__name__
                limit = 0 if 'DMA' in str(opc) or 'Collective' in str(opc) else max_waits
                if si is not None and si.on_wait and len(si.on_wait) > limit:
                    waits = si.on_wait
                    keep = waits[len(waits) - limit:] if limit else []
                    extra = waits[: len(waits) - limit]
                    si.on_wait = list(keep)
                    pos = i
                    for j in range(0, len(extra), max_waits):
                        chunk = extra[j:j + max_waits]
                        nop = mybir.InstNoOp(
                            name=f"{ins.name}-wsplit-{j}",
                            engine=ins.engine,
                            ins=[], outs=[],
                            sync_info=mybir.SyncInfo(on_wait=list(chunk), on_update=[]),
                        )
                        insts.insert(pos, nop)
                        pos += 1
                        i += 1
                        n += 1
                i += 1
    return n


# ---------------------------------------------------------------------------
def build_nc():
    nc = bass.Bass()

    def inp(name, shape):
        return nc.declare_dram_parameter(name, list(shape), F32, isOutput=False)

    wqkv = inp("wqkv", (NL, 128, 2048))
    wo = inp("wo", (NL, 128, 1024))
    wgu = inp("wgu", (NL, 128, 4096))
    wd = inp("wd", (NL, 128, 2048))
    wpl = inp("wpl", (NL, 128, 256))
    plew = inp("plew", (128, 7680))
    kt = inp("kt", (NL, 128, 2 * NP_))
    vv = inp("vv", (NL, NP_, 128))
    hidc = inp("hidc", (128, 8))
    rawc = inp("rawc", (128, 60))
    plenwc = inp("plenwc", (128, 60))
    ln1c = inp("ln1c", (128, 120))
    ln2c = inp("ln2c", (128, 120))
    cosq = inp("cosq", (128, 120))
    sinq = inp("sinq", (128, 120))
    cosk = inp("cosk", (128, 60))
    sink = inp("sink", (128, 60))
    xmask = inp("xmask", (128, 8))
    qma = inp("qma", (128, 8))
    qmb = inp("qmb", (128, 8))
    qhm = inp("qhm", (128, 8))
    vmask = inp("vmask", (128, 4))
    cmask8 = inp("cmask8", (128, 8))
    onescol = inp("onescol", (128, 1))
    onesrow = inp("onesrow", (1, 128))
    onesrow16 = inp("onesrow16", (1, 128))
    ident = inp("ident", (128, 128))

    h_out = nc.declare_dram_parameter("h_out", [128, 8], F32, isOutput=True)
    ple_out = nc.declare_dram_parameter("ple_out", [128, 60], F32, isOutput=True)
    k_out = nc.declare_dram_parameter("k_out", [NL, 128, 4], F32, isOutput=True)
    v_out = nc.declare_dram_parameter("v_out", [NL, 128, 4], F32, isOutput=True)

    with nc.allow_low_precision(reason="fp32r rounding of matmul operands is intentional"), \
         tile.TileContext(nc) as tc:
        _body(
            nc, tc,
            wqkv, wo, wgu, wd, wpl, plew, kt, vv, hidc, rawc, plenwc,
            ln1c, ln2c, cosq, sinq, cosk, sink,
            xmask, qma, qmb, qhm, vmask, cmask8,
            onescol, onesrow, onesrow16, ident,
            h_out, ple_out, k_out, v_out,
        )

    _split_excess_waits(nc)
    return nc


def _body(
    nc, tc,
    wqkv, wo, wgu, wd, wpl, plew, kt, vv, hidc, rawc, plenwc,
    ln1c, ln2c, cosq, sinq, cosk, sink,
    xmask, qma, qmb, qhm, vmask, cmask8,
    onescol, onesrow, onesrow16, ident,
    h_out, ple_out, k_out, v_out,
):
    mm = nc.tensor.matmul
    dve = nc.vector
    act = nc.scalar

    def mmr(out, lhsT, rhs, **kw):
        mm(out, lhsT, rhs, **kw)

    from contextlib import ExitStack

    stack = ExitStack()
    const = stack.enter_context(tc.tile_pool(name="const", bufs=1))
    state = stack.enter_context(tc.tile_pool(name="state", bufs=2))
    wpool = stack.enter_context(tc.tile_pool(name="wpool", bufs=3))
    spool = stack.enter_context(tc.tile_pool(name="spool", bufs=2))
    dram = stack.enter_context(tc.tile_pool(name="dram", bufs=2, space="DRAM"))
    psum = stack.enter_context(tc.tile_pool(name="psum", bufs=7, space="PSUM"))

    def pstile(shape, name):
        return psum.tile(list(shape), F32, tag="ps", name=name)

    def load_const(src, shape, tag):
        t = const.tile(list(shape), F32, tag=tag, name=tag)
        nc.sync.dma_start(out=t[:], in_=src[:])
        return t

    c_plew = load_const(plew, (128, 7680), "c_plew")
    c_hid = load_const(hidc, (128, 8), "c_hid")
    c_raw = load_const(rawc, (128, 60), "c_raw")
    c_plenw = load_const(plenwc, (128, 60), "c_plenw")
    c_ln1 = load_const(ln1c, (128, 120), "c_ln1")
    c_ln2 = load_const(ln2c, (128, 120), "c_ln2")
    c_cosq = load_const(cosq, (128, 120), "c_cosq")
    c_sinq = load_const(sinq, (128, 120), "c_sinq")
    c_cosk = load_const(cosk, (128, 60), "c_cosk")
    c_sink = load_const(sink, (128, 60), "c_sink")
    c_xm = load_const(xmask, (128, 8), "c_xm")
    c_qma = load_const(qma, (128, 8), "c_qma")
    c_qmb = load_const(qmb, (128, 8), "c_qmb")
    c_qhm = load_const(qhm, (128, 8), "c_qhm")
    c_vm = load_const(vmask, (128, 4), "c_vm")
    c_cm8 = load_const(cmask8, (128, 8), "c_cm8")
    c_one = load_const(onescol, (128, 1), "c_one")
    c_oner = load_const(onesrow, (1, 128), "c_oner")
    c_oner16 = load_const(onesrow16, (1, 128), "c_oner16")
    c_id = load_const(ident, (128, 128), "c_id")

    # ---- cross-core sum of a column-layout [128, F] SBUF tile ----
    ar_idx = [0]

    def allreduce(send_ap, F, tag):
        i = ar_idx[0]
        ar_idx[0] += 1
        inb = dram.tile([128, F], F32, tag=f"{tag}_in", name=f"{tag}_in{i}")
        outb = dram.tile(
            [8 * 128, F], F32, tag=f"{tag}_out", name=f"{tag}_out{i}",
            addr_space="Shared",
        )
        nc.scalar.dma_start(out=inb[:], in_=send_ap)
        nc.gpsimd.collective_compute(
            "AllGather",
            mybir.AluOpType.bypass,
            replica_groups=RG,
            ins=[inb.opt()],
            outs=[outb.opt()],
        )
        slots = spool.tile([128, 8 * F], F32, tag=f"{tag}_sl", name=f"{tag}_sl{i}")
        nc.scalar.dma_start(
            out=slots.rearrange("p (s f) -> p s f", s=8),
            in_=outb.rearrange("(s p) f -> p s f", p=128),
        )
        a = spool.tile([128, 4 * F], F32, tag=f"{tag}_a", name=f"{tag}_a{i}")
        dve.tensor_add(a[:], slots[:, : 4 * F], slots[:, 4 * F :])
        b = spool.tile([128, 2 * F], F32, tag=f"{tag}_b", name=f"{tag}_b{i}")
        dve.tensor_add(b[:], a[:, : 2 * F], a[:, 2 * F :])
        c = spool.tile([128, F], F32, tag=f"{tag}_c", name=f"{tag}_c{i}")
        dve.tensor_add(c[:], b[:, :F], b[:, F:])
        return c

    # ---- broadcast a [1, n] row of per-column scalars to [128, n] (SBUF) ----
    bc_idx = [0]

    def bcast_cols(row_ap, n, tag, ones=None):
        i = bc_idx[0]
        bc_idx[0] += 1
        t = pstile([128, n], f"bc_{tag}{i}")
        mm(t[:], (ones if ones is not None else c_oner)[:], row_ap,
           start=True, stop=True)
        s = spool.tile([128, n], F32, tag=f"bc_{tag}", name=f"bcs_{tag}{i}")
        dve.tensor_copy(s[:], t[:])
        return s

    # ---- rms scale for a [128, F] column-layout vector, D = 128*F ----
    rm_idx = [0]

    def global_rms_scale(v, F, D, tag):
        i = rm_idx[0]
        rm_idx[0] += 1
        sq = spool.tile([128, F], F32, tag=f"rm_sq_{tag}", name=f"rm_sq_{tag}{i}")
        dve.tensor_mul(sq[:], v[:], v[:])
        cs = pstile([1, F], f"rm_cs_{tag}{i}")
        mm(cs[:], c_one[:], sq[:], start=True, stop=True)
        tot = spool.tile([1, 1], F32, tag=f"rm_t_{tag}", name=f"rm_t_{tag}{i}")
        dve.reduce_sum(tot[:], cs[:], axis=AX.X)
        tt = spool.tile([1, 1], F32, tag=f"rm_tt_{tag}", name=f"rm_tt_{tag}{i}")
        dve.tensor_scalar(tt[:], tot[:], 1.0 / D, EPS,
                          op0=mybir.AluOpType.mult, op1=mybir.AluOpType.add)
        sr = spool.tile([1, 1], F32, tag=f"rm_s_{tag}", name=f"rm_s_{tag}{i}")
        act.activation(sr[:], tt[:], AF.Sqrt)
        rs = spool.tile([1, 1], F32, tag=f"rm_r_{tag}", name=f"rm_r_{tag}{i}")
        dve.reciprocal(rs[:], sr[:])
        scp = pstile([128, 1], f"rm_b_{tag}{i}")
        mm(scp[:], c_oner[:], rs[:], start=True, stop=True)
        scs = spool.tile([128, 1], F32, tag=f"rm_bs_{tag}", name=f"rm_bs_{tag}{i}")
        dve.tensor_copy(scs[:], scp[:])
        return scs

    # =========================== PLE stage ===============================
    xple = state.tile([128, 1], F32, tag="xple", name="xple")
    tmp8 = state.tile([128, 8], F32, tag="tmp8", name="tmp8")
    dve.tensor_mul(tmp8[:], c_hid[:], c_xm[:])
    dve.reduce_sum(xple[:], tmp8[:], axis=AX.X)

    pj = pstile([128, 60], "pj")
    for m in range(60):
        mmr(pj[:, m : m + 1], c_plew[:, 128 * m : 128 * (m + 1)], xple[:],
            start=True, stop=True)
    pjs = state.tile([128, 60], F32, tag="pjs", name="pjs")
    dve.tensor_copy(pjs[:], pj[:])
    projs = allreduce(pjs[:], 60, "arp")

    # group-rms over 30 groups of 256 (cols 2g, 2g+1)
    psq = state.tile([128, 60], F32, tag="psq", name="psq")
    dve.tensor_mul(psq[:], projs[:], projs[:])
    pcs = pstile([1, 60], "pcs")
    mm(pcs[:], c_one[:], psq[:], start=True, stop=True)
    pcss = state.tile([1, 60], F32, tag="pcss", name="pcss")
    dve.tensor_copy(pcss[:], pcs[:])
    g2 = state.tile([1, 30], F32, tag="g2", name="g2")
    dve.tensor_add(g2[:], pcss[:, 0::2], pcss[:, 1::2])
    g2e = state.tile([1, 30], F32, tag="g2e", name="g2e")
    dve.tensor_scalar(g2e[:], g2[:], 1.0 / PLE, EPS,
                      op0=mybir.AluOpType.mult, op1=mybir.AluOpType.add)
    gsr = state.tile([1, 30], F32, tag="gsr", name="gsr")
    act.activation(gsr[:], g2e[:], AF.Sqrt)
    grs = state.tile([1, 30], F32, tag="grs", name="grs")
    dve.reciprocal(grs[:], gsr[:])
    s60 = state.tile([1, 60], F32, tag="s60", name="s60")
    dve.tensor_copy(s60[:, 0::2], grs[:])
    dve.tensor_copy(s60[:, 1::2], grs[:])
    sc60 = bcast_cols(s60[:], 60, "p60")
    normed = state.tile([128, 60], F32, tag="normed", name="normed")
    dve.tensor_mul(normed[:], projs[:], sc60[:])
    # plenw and raw are pre-scaled by 2^-0.5 on host
    plec = state.tile([128, 60], F32, tag="plec", name="plec")
    dve.tensor_mul(plec[:], normed[:], c_plenw[:])
    dve.tensor_add(plec[:], plec[:], c_raw[:])
    nc.sync.dma_start(out=ple_out[:], in_=plec[:])

    # =========================== layers ==================================
    h = state.tile([128, 8], F32, tag="h", name="h0")
    dve.tensor_copy(h[:], c_hid[:])

    for l in range(NL):
        w_qkv = wpool.tile([128, 2048], F32, tag="w_qkv", name=f"w_qkv{l}")
        nc.sync.dma_start(out=w_qkv[:], in_=wqkv[l])
        w_o = wpool.tile([128, 1024], F32, tag="w_o", name=f"w_o{l}")
        nc.sync.dma_start(out=w_o[:], in_=wo[l])
        w_gu = wpool.tile([128, 4096], F32, tag="w_gu", name=f"w_gu{l}")
        nc.sync.dma_start(out=w_gu[:], in_=wgu[l])
        w_d = wpool.tile([128, 2048], F32, tag="w_d", name=f"w_d{l}")
        nc.sync.dma_start(out=w_d[:], in_=wd[l])
        w_pl = wpool.tile([128, 256], F32, tag="w_pl", name=f"w_pl{l}")
        nc.sync.dma_start(out=w_pl[:], in_=wpl[l])
        t_kt = wpool.tile([128, 2 * NP_], F32, tag="t_kt", name=f"t_kt{l}")
        nc.sync.dma_start(out=t_kt[:], in_=kt[l])
        vsb = wpool.tile([NP_, 128], F32, tag="vsb", name=f"vsb{l}")
        nc.sync.dma_start(out=vsb[:], in_=vv[l])

        # ---- x = rms(h) * ln1 ----
        hsc = global_rms_scale(h, 8, H, "h1")
        x = spool.tile([128, 8], F32, tag="x", name=f"x{l}")
        dve.tensor_scalar_mul(x[:], h[:], hsc[:])

        # ---- qkv partials ----
        xc8 = spool.tile([128, 8], F32, tag="xc8", name=f"xc8{l}")
        dve.tensor_mul(xc8[:], x[:], c_xm[:])
        xcs = spool.tile([128, 1], F32, tag="xcs", name=f"xcs{l}")
        dve.reduce_sum(xcs[:], xc8[:], axis=AX.X)
        pqkv = pstile([128, 16], f"pqkv{l}")
        for m in range(16):
            mmr(pqkv[:, m : m + 1], w_qkv[:, 128 * m : 128 * (m + 1)], xcs[:],
                start=True, stop=True)
        sqkv = spool.tile([128, 16], F32, tag="sqkv", name=f"sqkv{l}")
        dve.tensor_copy(sqkv[:], pqkv[:])
        qkvc = allreduce(sqkv[:], 16, "ar1")
        q = qkvc[:, 0:8]
        k = qkvc[:, 8:12]
        v = qkvc[:, 12:16]

        # v straight to cache output
        nc.sync.dma_start(out=v_out[l], in_=v)

        # ---- q rms (with 1/16 folded) + rope ----
        qksq = spool.tile([128, 12], F32, tag="qksq", name=f"qksq{l}")
        dve.tensor_mul(qksq[:], qkvc[:, 0:12], qkvc[:, 0:12])
        qkcs = pstile([1, 12], f"qkcs{l}")
        mm(qkcs[:], c_one[:], qksq[:], start=True, stop=True)
        qkss = spool.tile([1, 12], F32, tag="qkss", name=f"qkss{l}")
        dve.tensor_copy(qkss[:], qkcs[:])
        qg = spool.tile([1, 4], F32, tag="qg", name=f"qg{l}")
        dve.tensor_add(qg[:], qkss[:, 0:8:2], qkss[:, 1:8:2])
        qge = spool.tile([1, 4], F32, tag="qge", name=f"qge{l}")
        dve.tensor_scalar(qge[:], qg[:], 256.0 / HD, 256.0 * EPS,
                          op0=mybir.AluOpType.mult, op1=mybir.AluOpType.add)
        qsr = spool.tile([1, 4], F32, tag="qsr", name=f"qsr{l}")
        act.activation(qsr[:], qge[:], AF.Sqrt)
        qrs = spool.tile([1, 4], F32, tag="qrs", name=f"qrs{l}")
        dve.reciprocal(qrs[:], qsr[:])
        qs8 = spool.tile([1, 8], F32, tag="qs8", name=f"qs8{l}")
        dve.tensor_copy(qs8[:, 0::2], qrs[:])
        dve.tensor_copy(qs8[:, 1::2], qrs[:])
        qsc = bcast_cols(qs8[:], 8, "q")  # rsqrt/16 folded via 256x sqrt arg
        qn_ = spool.tile([128, 8], F32, tag="qn_", name=f"qn_{l}")
        dve.tensor_mul(qn_[:], q, qsc[:])
        rotq = spool.tile([128, 8], F32, tag="rotq", name=f"rotq{l}")
        dve.tensor_scalar_mul(rotq[:, 0::2], qn_[:, 1::2], -1.0)
        dve.tensor_copy(rotq[:, 1::2], qn_[:, 0::2])
        qr = spool.tile([128, 8], F32, tag="qr", name=f"qr{l}")
        dve.tensor_mul(qr[:], qn_[:], c_cosq[:, 8 * l : 8 * l + 8])
        dve.tensor_mul(rotq[:], rotq[:], c_sinq[:, 8 * l : 8 * l + 8])
        dve.tensor_add(qr[:], qr[:], rotq[:])

        # ---- k rms + rope ----
        kg = spool.tile([1, 2], F32, tag="kg", name=f"kg{l}")
        dve.tensor_add(kg[:], qkss[:, 8:12:2], qkss[:, 9:12:2])
        kge = spool.tile([1, 2], F32, tag="kge", name=f"kge{l}")
        dve.tensor_scalar(kge[:], kg[:], 1.0 / HD, EPS,
                          op0=mybir.AluOpType.mult, op1=mybir.AluOpType.add)
        ksr = spool.tile([1, 2], F32, tag="ksr", name=f"ksr{l}")
        act.activation(ksr[:], kge[:], AF.Sqrt)
        krs = spool.tile([1, 2], F32, tag="krs", name=f"krs{l}")
        dve.reciprocal(krs[:], ksr[:])
        ks4 = spool.tile([1, 4], F32, tag="ks4", name=f"ks4{l}")
        dve.tensor_copy(ks4[:, 0::2], krs[:])
        dve.tensor_copy(ks4[:, 1::2], krs[:])
        ksc = bcast_cols(ks4[:], 4, "k")
        dwe = spool.tile([1, 1], F32, tag="dwe", name=f"dwe{l}")
        act.activation(dwe[:], krs[:, 0:1], AF.Exp)  # pre-warm Exp table
        kn_ = spool.tile([128, 4], F32, tag="kn_", name=f"kn_{l}")
        dve.tensor_mul(kn_[:], k, ksc[:])
        rotk = spool.tile([128, 4], F32, tag="rotk", name=f"rotk{l}")
        dve.tensor_scalar_mul(rotk[:, 0::2], kn_[:, 1::2], -1.0)
        dve.tensor_copy(rotk[:, 1::2], kn_[:, 0::2])
        kr = spool.tile([128, 4], F32, tag="kr", name=f"kr{l}")
        dve.tensor_mul(kr[:], kn_[:], c_cosk[:, 4 * l : 4 * l + 4])
        dve.tensor_mul(rotk[:], rotk[:], c_sink[:, 4 * l : 4 * l + 4])
        dve.tensor_add(kr[:], kr[:], rotk[:])
        nc.sync.dma_start(out=k_out[l], in_=kr[:])

        # ---- my q head halves ----
        tq = spool.tile([128, 8], F32, tag="tq", name=f"tq{l}")
        dve.tensor_mul(tq[:], qr[:], c_qma[:])
        qa = spool.tile([128, 1], F32, tag="qa", name=f"qa{l}")
        dve.reduce_sum(qa[:], tq[:], axis=AX.X)
        dve.tensor_mul(tq[:], qr[:], c_qmb[:])
        qb = spool.tile([128, 1], F32, tag="qb", name=f"qb{l}")
        dve.reduce_sum(qb[:], tq[:], axis=AX.X)

        # ---- scores over cache rows 0..99 ----
        psc = pstile([1, POS + 1], f"psc{l}")
        mm(psc[:, 0:NP_], qa[:], t_kt[:, 0:NP_], start=True, stop=False)
        mm(psc[:, 0:NP_], qb[:], t_kt[:, NP_ : 2 * NP_], start=False, stop=True)

        # ---- score at row 100 (new k) ----
        krep = spool.tile([128, 8], F32, tag="krep", name=f"krep{l}")
        kr_b = (kr.rearrange("p (u e) -> p u e", e=2)
                .broadcast_to((128, 2, 2, 2))
                .rearrange("p u e r -> p u r e"))
        dve.tensor_mul(
            krep.rearrange("p (u r e) -> p u r e", u=2, r=2, e=2),
            qr.rearrange("p (u r e) -> p u r e", u=2, r=2, e=2),
            kr_b)
        dve.tensor_mul(krep[:], krep[:], c_qhm[:])
        s1cs = pstile([1, 8], f"s1cs{l}")
        mm(s1cs[:], c_one[:], krep[:], start=True, stop=True)
        s100 = spool.tile([1, 1], F32, tag="s100", name=f"s100{l}")
        dve.reduce_sum(s100[:], s1cs[:], axis=AX.X)

        sc = spool.tile([1, POS + 1], F32, tag="sc", name=f"sc{l}")
        dve.tensor_copy(sc[:, 0:NP_], psc[:, 0:NP_])
        dve.tensor_copy(sc[:, POS : POS + 1], s100[:])

        # ---- softmax ----
        mx = spool.tile([1, 1], F32, tag="mx", name=f"mx{l}")
        dve.reduce_max(mx[:], sc[:], axis=AX.X)
        nmx = spool.tile([1, 1], F32, tag="nmx", name=f"nmx{l}")
        dve.tensor_scalar_mul(nmx[:], mx[:], -1.0)
        e = spool.tile([1, POS + 1], F32, tag="e", name=f"e{l}")
        esum = spool.tile([1, 1], F32, tag="esum", name=f"esum{l}")
        act.activation(e[:], sc[:], AF.Exp, bias=nmx[:], scale=1.0,
                       accum_out=esum[:])
        dws = spool.tile([1, 1], F32, tag="dws", name=f"dws{l}")
        act.activation(dws[:], esum[:], AF.Sqrt)  # pre-warm Sqrt table
        rcp = spool.tile([1, 1], F32, tag="rcp", name=f"rcp{l}")
        dve.reciprocal(rcp[:], esum[:])
        p = spool.tile([1, POS + 1], F32, tag="p", name=f"p{l}")
        dve.tensor_scalar_mul(p[:], e[:], rcp[:])

        # ---- p[:100] to column; v_new row ----
        pct = pstile([NP_, 1], f"pct{l}")
        nc.tensor.transpose(pct[:], p[:, 0:NP_], c_id[0:1, 0:1])
        pcol = spool.tile([NP_, 1], F32, tag="pcol", name=f"pcol{l}")
        dve.tensor_copy(pcol[:], pct[:])

        tv8 = spool.tile([128, 4], F32, tag="tv8", name=f"tv8{l}")
        dve.tensor_mul(tv8[:], v, c_vm[:])
        vsl = spool.tile([128, 1], F32, tag="vsl", name=f"vsl{l}")
        dve.reduce_sum(vsl[:], tv8[:], axis=AX.X)
        vrt = pstile([1, 128], f"vrt{l}")
        nc.tensor.transpose(vrt[:], vsl[:], c_id[:])
        vrow = spool.tile([1, 128], F32, tag="vrow", name=f"vrow{l}")
        dve.tensor_copy(vrow[:], vrt[:])

        # ---- att = V[0:100]^T p[0:100] + p[100] * v_new ----
        patt = pstile([128, 1], f"patt{l}")
        mm(patt[:], vsb[:], pcol[:], start=True, stop=False)
        mm(patt[:], vrow[:], p[:, POS : POS + 1], start=False, stop=True)
        attc = spool.tile([128, 1], F32, tag="attc", name=f"attc{l}")
        dve.tensor_copy(attc[:], patt[:])

        # ---- o partials ----
        po = pstile([128, 8], f"po{l}")
        for m in range(8):
            mmr(po[:, m : m + 1], w_o[:, 128 * m : 128 * (m + 1)], attc[:],
                start=True, stop=True)
        so = spool.tile([128, 8], F32, tag="so", name=f"so{l}")
        dve.tensor_copy(so[:], po[:])
        oc = allreduce(so[:], 8, "ar2")
        h2 = state.tile([128, 8], F32, tag="h2", name=f"h2_{l}")
        dve.tensor_add(h2[:], h[:], oc[:])

        # ---- x2 = rms(h2) * ln2 ----
        h2sc = global_rms_scale(h2, 8, H, "h2")
        dwg = spool.tile([1, 1], F32, tag="dwg", name=f"dwg{l}")
        act.activation(dwg[:], h2sc[0:1, :], AF.Gelu_apprx_tanh)  # pre-warm
        x2 = spool.tile([128, 8], F32, tag="x2", name=f"x2{l}")
        dve.tensor_scalar_mul(x2[:], h2[:], h2sc[:])

        # ---- mlp g/u (col shard: full x2 contraction) ----
        pg = pstile([128, 2], f"pg{l}")
        pu = pstile([128, 2], f"pu{l}")
        for m in range(2):
            for kk in range(8):
                mmr(pg[:, m : m + 1],
                    w_gu[:, 256 * kk + 128 * m : 256 * kk + 128 * m + 128],
                    x2[:, kk : kk + 1], start=(kk == 0), stop=(kk == 7))
        for m in range(2):
            for kk in range(8):
                mmr(pu[:, m : m + 1],
                    w_gu[:, 2048 + 256 * kk + 128 * m : 2048 + 256 * kk + 128 * m + 128],
                    x2[:, kk : kk + 1], start=(kk == 0), stop=(kk == 7))
        ga = spool.tile([128, 2], F32, tag="ga", name=f"ga{l}")
        act.activation(ga[:], pg[:], AF.Gelu_apprx_tanh)
        mc = spool.tile([128, 2], F32, tag="mc", name=f"mc{l}")
        dve.tensor_mul(mc[:], ga[:], pu[:])
        dwq = spool.tile([1, 1], F32, tag="dwq", name=f"dwq{l}")
        act.activation(dwq[:], ga[0:1, 0:1], AF.Sqrt)  # pre-warm Sqrt table

        # ---- d partials (row shard of Wd over my 256 dff dims) ----
        pd = pstile([128, 8], f"pd{l}")
        for m in range(8):
            for kk in range(2):
                mmr(pd[:, m : m + 1],
                    w_d[:, 1024 * kk + 128 * m : 1024 * kk + 128 * m + 128],
                    mc[:, kk : kk + 1], start=(kk == 0), stop=(kk == 1))
        # ---- ple contribution (col shard of Wpl; placed by col mask) ----
        ppl = pstile([128, 1], f"ppl{l}")
        for kk in range(2):
            mmr(ppl[:], w_pl[:, 128 * kk : 128 * kk + 128],
                plec[:, 2 * l + kk : 2 * l + kk + 1],
                start=(kk == 0), stop=(kk == 1))
        plp = spool.tile([128, 1], F32, tag="plp", name=f"plp{l}")
        dve.tensor_copy(plp[:], ppl[:])
        sd = spool.tile([128, 8], F32, tag="sd", name=f"sd{l}")
        dve.tensor_scalar_mul(sd[:], c_cm8[:], plp[:])
        dve.tensor_add(sd[:], sd[:], pd[:])

        dc = allreduce(sd[:], 8, "ar3")
        hn = state.tile([128, 8], F32, tag="h", name=f"h{l + 1}")
        dve.tensor_add(hn[:], h2[:], dc[:])
        h = hn

    nc.sync.dma_start(out=h_out[:], in_=h[:])

    stack.close()


# ---------------------------------------------------------------------------
# host-side shard prep
def _col(vec, parts=128):
    """[D] -> [128, D/128] column layout (col f = dims 128f..128f+128)."""
    v = np.asarray(vec, dtype=np.float32).reshape(-1)
    return np.ascontiguousarray(v.reshape(-1, parts).T)


def _uncol(mat):
    return np.ascontiguousarray(mat.T).reshape(-1)


def _host_prep(inp):
    f32 = np.float32
    ln1w = np.asarray(inp["ln1"], f32)[:, :, None]
    ln2w = np.asarray(inp["ln2"], f32)[:, :, None]
    Wq = np.asarray(inp["Wq"], f32) * ln1w
    Wk = np.asarray(inp["Wk"], f32) * ln1w
    Wv = np.asarray(inp["Wv"], f32) * ln1w
    Wo = np.asarray(inp["Wo"], f32)
    Wg = np.asarray(inp["Wg"], f32) * ln2w
    Wu = np.asarray(inp["Wu"], f32) * ln2w
    Wd = np.asarray(inp["Wd"], f32)
    Wpl = np.asarray(inp["Wpl"], f32)
    plw = np.asarray(inp["ple_proj_W"], f32)
    qn = np.asarray(inp["qn"], f32)
    kn = np.asarray(inp["kn"], f32)
    ln1 = np.asarray(inp["ln1"], f32)
    ln2 = np.asarray(inp["ln2"], f32)
    Ks = np.asarray(inp["K_sliding_in"], f32)
    Vs = np.asarray(inp["V_sliding_in"], f32)
    Kf = np.asarray(inp["K_full_in"], f32)
    Vf = np.asarray(inp["V_full_in"], f32)
    cos_s = np.asarray(inp["cos_s"], f32).reshape(HD)
    sin_s = np.asarray(inp["sin_s"], f32).reshape(HD)
    cos_f = np.asarray(inp["cos_f"], f32).reshape(HD)
    sin_f = np.asarray(inp["sin_f"], f32).reshape(HD)
    hid = np.asarray(inp["hidden_states"], f32).reshape(H)
    raw = np.asarray(inp["per_layer_raw"], f32).reshape(NTOT * PLE)
    plnw = np.asarray(inp["ple_norm_w"], f32).reshape(PLE)

    # per-layer cache K/V (sliding/full), rows 0..99 of the right kv slice
    kcache = []
    vcache = []
    si = fi = 0
    for l in range(NL):
        if l in FULL_LAYERS:
            kcache.append(Kf[fi, 0])
            vcache.append(Vf[fi, 0])
            fi += 1
        else:
            kcache.append(Ks[si, 0])
            vcache.append(Vs[si, 0])
            si += 1

    # per-layer cos/sin columns with qn/kn folded
    cosq_h = np.zeros((128, 120), f32)
    sinq_h = np.zeros((128, 120), f32)
    cosk_h = np.zeros((128, 60), f32)
    sink_h = np.zeros((128, 60), f32)
    ln1_h = np.zeros((128, 120), f32)
    ln2_h = np.zeros((128, 120), f32)
    for l in range(NL):
        cos = cos_f if l in FULL_LAYERS else cos_s
        sin = sin_f if l in FULL_LAYERS else sin_s
        qnl, knl = qn[l], kn[l]
        qn_sw = np.concatenate([qnl[128:], qnl[:128]])
        kn_sw = np.concatenate([knl[128:], knl[:128]])
        cq = _col(cos * qnl)      # [128,2]
        sq_ = _col(sin * qn_sw)
        ck = _col(cos * knl)
        sk = _col(sin * kn_sw)
        for u in range(NH):
            cosq_h[:, 8 * l + 2 * u : 8 * l + 2 * u + 2] = cq
            sinq_h[:, 8 * l + 2 * u : 8 * l + 2 * u + 2] = sq_
        for u in range(NKV):
            cosk_h[:, 4 * l + 2 * u : 4 * l + 2 * u + 2] = ck
            sink_h[:, 4 * l + 2 * u : 4 * l + 2 * u + 2] = sk
        ln1_h[:, 8 * l : 8 * l + 8] = _col(ln1[l])
        ln2_h[:, 8 * l : 8 * l + 8] = _col(ln2[l])

    common = dict(
        hidc=_col(hid),
        rawc=_col(raw) * np.float32(2 ** -0.5),
        plenwc=np.tile(_col(plnw), (1, 30)).astype(f32) * np.float32(2 ** -0.5),
        ln1c=ln1_h, ln2c=ln2_h,
        cosq=cosq_h, sinq=sinq_h, cosk=cosk_h, sink=sink_h,
        onescol=np.ones((128, 1), f32),
        onesrow=np.ones((1, 128), f32),
        onesrow16=np.full((1, 128), HD ** -0.5, f32),
        ident=np.eye(128, dtype=f32),
    )

    in_maps = []
    for c in range(N_CORES):
        r0 = 128 * c
        j = c // 2          # q head
        half = c % 2
        jk = c // 4         # kv head
        wqkv_c = np.concatenate(
            [Wq[:, r0 : r0 + 128, :], Wk[:, r0 : r0 + 128, :],
             Wv[:, r0 : r0 + 128, :]], axis=2)
        worow = 256 * j + 128 * half
        wo_c = Wo[:, worow : worow + 128, :]
        gcols = slice(256 * c, 256 * c + 256)
        wg_c = (Wg[:, :, gcols].reshape(NL, 8, 128, 256)
                .transpose(0, 2, 1, 3).reshape(NL, 128, 2048))
        wu_c = (Wu[:, :, gcols].reshape(NL, 8, 128, 256)
                .transpose(0, 2, 1, 3).reshape(NL, 128, 2048))
        wgu_c = np.concatenate([wg_c, wu_c], axis=2)
        wd_c = (Wd[:, 256 * c : 256 * c + 256, :].reshape(NL, 2, 128, 1024)
                .transpose(0, 2, 1, 3).reshape(NL, 128, 2048))
        wpl_c = (Wpl[:, :, 128 * c : 128 * c + 128].reshape(NL, 2, 128, 128)
                 .transpose(0, 2, 1, 3).reshape(NL, 128, 256))
        plew_c = plw[r0 : r0 + 128, :] * np.float32(H ** -0.5)

        kt_c = np.zeros((NL, 128, 2 * NP_), f32)
        vv_c = np.zeros((NL, NP_, 128), f32)
        for l in range(NL):
            kk = kcache[l][0:NP_, 256 * jk : 256 * jk + 256]  # [100,256]
            ktt = np.ascontiguousarray(kk.T).reshape(2, 128, NP_)
            kt_c[l, :, 0:NP_] = ktt[0]
            kt_c[l, :, NP_ : 2 * NP_] = ktt[1]
            vcol0 = 256 * jk + 128 * half
            vv_c[l] = vcache[l][0:NP_, vcol0 : vcol0 + 128]

        def onehot(ncols, hot):
            m = np.zeros((128, ncols), f32)
            for hh in (hot if isinstance(hot, (list, tuple)) else [hot]):
                m[:, hh] = 1.0
            return m

        im = dict(common)
        im.update(
            wqkv=np.ascontiguousarray(wqkv_c),
            wo=np.ascontiguousarray(wo_c),
            wgu=np.ascontiguousarray(wgu_c),
            wd=np.ascontiguousarray(wd_c),
            wpl=np.ascontiguousarray(wpl_c),
            plew=np.ascontiguousarray(plew_c),
            kt=kt_c,
            vv=vv_c,
            xmask=onehot(8, c),
            qma=onehot(8, 2 * j),
            qmb=onehot(8, 2 * j + 1),
            qhm=onehot(8, [2 * j, 2 * j + 1]),
            vmask=onehot(4, 2 * jk + half),
            cmask8=onehot(8, c),
        )
        in_maps.append(im)
    return in_maps


_CACHED_NC = None


def _get_nc():
    global _CACHED_NC
    if _CACHED_NC is None:
        _CACHED_NC = build_nc()
    return _CACHED_NC


def kernel(_trace=False, **inputs):
    nc = _get_nc()
    in_maps = _host_prep(inputs)
    res = run_bass_kernel_spmd(nc, in_maps, CORE_IDS, trace=_trace)
    r0 = res.results[0]

    h = _uncol(r0["h_out"]).reshape(1, 1, H)
    ple = _uncol(r0["ple_out"]).reshape(1, 1, NTOT * PLE)
    k_new = np.stack([_uncol(r0["k_out"][l]) for l in range(NL)])  # [15,512]
    v_new = np.stack([_uncol(r0["v_out"][l]) for l in range(NL)])

    Ks = np.array(np.asarray(inputs["K_sliding_in"], np.float32), copy=True)
    Vs = np.array(np.asarray(inputs["V_sliding_in"], np.float32), copy=True)
    Kf = np.array(np.asarray(inputs["K_full_in"], np.float32), copy=True)
    Vf = np.array(np.asarray(inputs["V_full_in"], np.float32), copy=True)
    si = fi = 0
    kv13 = kv14 = None
    for l in range(NL):
        if l in FULL_LAYERS:
            Kf[fi, 0, POS, :] = k_new[l]
            Vf[fi, 0, POS, :] = v_new[l]
            fi += 1
        else:
            Ks[si, 0, POS, :] = k_new[l]
            Vs[si, 0, POS, :] = v_new[l]
            si += 1
    kv13_k = k_new[13][:HD].reshape(1, 1, 1, HD)
    kv13_v = v_new[13][:HD].reshape(1, 1, 1, HD)
    kv14_k = k_new[14].reshape(1, 1, 1, NKV * HD)
    kv14_v = v_new[14].reshape(1, 1, 1, NKV * HD)

    out = (h, Ks, Vs, Kf, Vf, kv13_k, kv13_v, kv14_k, kv14_v, ple)
    if _trace:
        return out, res
    return out


# revision 23
# speedup vs baseline: 1.0354x; 1.0047x over previous
"""Trainium2 Bass kernel for nn_MergedChunk12 (15-layer decode step, TP-8).

Sharding: every weight matmul is 8-way sharded so each weight byte is read
by exactly one core; partial sums are combined with an AllGather of
column-layout [128,F] tiles followed by a local 3-op tree fold (= AllReduce).
Attention (101 live positions at POS=100) is sharded per Q-head-half.
KV-cache passthrough (row-100 scatter) is done host-side.

Vector layout convention on device: a D-vector lives as [128, D/128] f32,
column f holding dims [128f, 128(f+1)).
"""

import numpy as np

import concourse.bass as bass
import concourse.mybir as mybir
from concourse import tile
from concourse.bass_utils import run_bass_kernel_spmd

F32 = mybir.dt.float32
F32R = mybir.dt.float32r
AF = mybir.ActivationFunctionType
AX = mybir.AxisListType

# model dims
H, NH, NKV, HD = 1024, 4, 2, 256
DFF, NL, NTOT, PLE = 2048, 15, 30, 256
W, CTX = 1024, 4096
FULL_LAYERS = (4, 9, 14)
POS = 100
EPS = 1e-6
NP_ = 100  # live cache rows (0..POS-1); row POS comes from the new k/v

N_CORES = 8
CORE_IDS = list(range(N_CORES))
RG = [CORE_IDS]


# ---------------------------------------------------------------------------
# walrus in this container rejects >1 sync-wait per instruction: hoist extras
# onto preceding NOPs on the same engine.
def _split_excess_waits(nc, max_waits=1):
    n = 0
    for fn in nc.m.functions:
        for bb in fn.blocks:
            i = 0
            insts = bb.instructions
            while i < len(insts):
                ins = insts[i]
                si = getattr(ins, 'sync_info', None)
                opc = getattr(ins, 'opcode', '') or type(ins).__name__
                limit = 0 if 'DMA' in str(opc) or 'Collective' in str(opc) else max_waits
                if si is not None and si.on_wait and len(si.on_wait) > limit:
                    waits = si.on_wait
                    keep = waits[len(waits) - limit:] if limit else []
                    extra = waits[: len(waits) - limit]
                    si.on_wait = list(keep)
                    pos = i
                    for j in range(0, len(extra), max_waits):
                        chunk = extra[j:j + max_waits]
                        nop = mybir.InstNoOp(
                            name=f"{ins.name}-wsplit-{j}",
                            engine=ins.engine,
                            ins=[], outs=[],
                            sync_info=mybir.SyncInfo(on_wait=list(chunk), on_update=[]),
                        )
                        insts.insert(pos, nop)
                        pos += 1
                        i += 1
                        n += 1
                i += 1
    return n


# ---------------------------------------------------------------------------
def build_nc():
    nc = bass.Bass()

    def inp(name, shape):
        return nc.declare_dram_parameter(name, list(shape), F32, isOutput=False)

    wqkv = inp("wqkv", (NL, 128, 2048))
    wo = inp("wo", (NL, 128, 1024))
    wgu = inp("wgu", (NL, 128, 4096))
    wd = inp("wd", (NL, 128, 2048))
    wpl = inp("wpl", (NL, 128, 256))
    plew = inp("plew", (128, 7680))
    kt = inp("kt", (NL, 128, 2 * NP_))
    vv = inp("vv", (NL, NP_, 128))
    hidc = inp("hidc", (128, 8))
    rawc = inp("rawc", (128, 60))
    plenwc = inp("plenwc", (128, 60))
    ln1c = inp("ln1c", (128, 120))
    ln2c = inp("ln2c", (128, 120))
    cosq = inp("cosq", (128, 120))
    sinq = inp("sinq", (128, 120))
    cosk = inp("cosk", (128, 60))
    sink = inp("sink", (128, 60))
    xmask = inp("xmask", (128, 8))
    qma = inp("qma", (128, 8))
    qmb = inp("qmb", (128, 8))
    qhm = inp("qhm", (128, 8))
    vmask = inp("vmask", (128, 4))
    cmask8 = inp("cmask8", (128, 8))
    onescol = inp("onescol", (128, 1))
    onesrow = inp("onesrow", (1, 128))
    onesrow16 = inp("onesrow16", (1, 128))
    ident = inp("ident", (128, 128))

    h_out = nc.declare_dram_parameter("h_out", [128, 8], F32, isOutput=True)
    ple_out = nc.declare_dram_parameter("ple_out", [128, 60], F32, isOutput=True)
    k_out = nc.declare_dram_parameter("k_out", [NL, 128, 4], F32, isOutput=True)
    v_out = nc.declare_dram_parameter("v_out", [NL, 128, 4], F32, isOutput=True)

    with nc.allow_low_precision(reason="fp32r rounding of matmul operands is intentional"), \
         tile.TileContext(nc) as tc:
        _body(
            nc, tc,
            wqkv, wo, wgu, wd, wpl, plew, kt, vv, hidc, rawc, plenwc,
            ln1c, ln2c, cosq, sinq, cosk, sink,
            xmask, qma, qmb, qhm, vmask, cmask8,
            onescol, onesrow, onesrow16, ident,
            h_out, ple_out, k_out, v_out,
        )

    _split_excess_waits(nc)
    return nc


def _body(
    nc, tc,
    wqkv, wo, wgu, wd, wpl, plew, kt, vv, hidc, rawc, plenwc,
    ln1c, ln2c, cosq, sinq, cosk, sink,
    xmask, qma, qmb, qhm, vmask, cmask8,
    onescol, onesrow, onesrow16, ident,
    h_out, ple_out, k_out, v_out,
):
    mm = nc.tensor.matmul
    dve = nc.vector
    act = nc.scalar

    def mmr(out, lhsT, rhs, **kw):
        mm(out, lhsT, rhs, **kw)

    from contextlib import ExitStack

    stack = ExitStack()
    const = stack.enter_context(tc.tile_pool(name="const", bufs=1))
    state = stack.enter_context(tc.tile_pool(name="state", bufs=2))
    wpool = stack.enter_context(tc.tile_pool(name="wpool", bufs=3))
    spool = stack.enter_context(tc.tile_pool(name="spool", bufs=2))
    dram = stack.enter_context(tc.tile_pool(name="dram", bufs=2, space="DRAM"))
    psum = stack.enter_context(tc.tile_pool(name="psum", bufs=7, space="PSUM"))

    def pstile(shape, name):
        return psum.tile(list(shape), F32, tag="ps", name=name)

    def load_const(src, shape, tag):
        t = const.tile(list(shape), F32, tag=tag, name=tag)
        nc.sync.dma_start(out=t[:], in_=src[:])
        return t

    c_plew = load_const(plew, (128, 7680), "c_plew")
    c_hid = load_const(hidc, (128, 8), "c_hid")
    c_raw = load_const(rawc, (128, 60), "c_raw")
    c_plenw = load_const(plenwc, (128, 60), "c_plenw")
    c_cosq = load_const(cosq, (128, 120), "c_cosq")
    c_sinq = load_const(sinq, (128, 120), "c_sinq")
    c_cosk = load_const(cosk, (128, 60), "c_cosk")
    c_sink = load_const(sink, (128, 60), "c_sink")
    c_xm = load_const(xmask, (128, 8), "c_xm")
    c_qma = load_const(qma, (128, 8), "c_qma")
    c_qmb = load_const(qmb, (128, 8), "c_qmb")
    c_qhm = load_const(qhm, (128, 8), "c_qhm")
    c_vm = load_const(vmask, (128, 4), "c_vm")
    c_cm8 = load_const(cmask8, (128, 8), "c_cm8")
    c_one = load_const(onescol, (128, 1), "c_one")
    c_oner = load_const(onesrow, (1, 128), "c_oner")
    c_oner16 = load_const(onesrow16, (1, 128), "c_oner16")
    c_id = load_const(ident, (128, 128), "c_id")

    # ---- cross-core sum of a column-layout [128, F] SBUF tile ----
    ar_idx = [0]

    def allreduce(send_ap, F, tag):
        i = ar_idx[0]
        ar_idx[0] += 1
        inb = dram.tile([128, F], F32, tag=f"{tag}_in", name=f"{tag}_in{i}")
        outb = dram.tile(
            [8 * 128, F], F32, tag=f"{tag}_out", name=f"{tag}_out{i}",
            addr_space="Shared",
        )
        nc.scalar.dma_start(out=inb[:], in_=send_ap)
        nc.gpsimd.collective_compute(
            "AllGather",
            mybir.AluOpType.bypass,
            replica_groups=RG,
            ins=[inb.opt()],
            outs=[outb.opt()],
        )
        slots = spool.tile([128, 8 * F], F32, tag=f"{tag}_sl", name=f"{tag}_sl{i}")
        nc.scalar.dma_start(
            out=slots.rearrange("p (s f) -> p s f", s=8),
            in_=outb.rearrange("(s p) f -> p s f", p=128),
        )
        a = spool.tile([128, 4 * F], F32, tag=f"{tag}_a", name=f"{tag}_a{i}")
        dve.tensor_add(a[:], slots[:, : 4 * F], slots[:, 4 * F :])
        b = spool.tile([128, 2 * F], F32, tag=f"{tag}_b", name=f"{tag}_b{i}")
        dve.tensor_add(b[:], a[:, : 2 * F], a[:, 2 * F :])
        c = spool.tile([128, F], F32, tag=f"{tag}_c", name=f"{tag}_c{i}")
        dve.tensor_add(c[:], b[:, :F], b[:, F:])
        return c

    # ---- broadcast a [1, n] row of per-column scalars to [128, n] (SBUF) ----
    bc_idx = [0]

    def bcast_cols(row_ap, n, tag, ones=None):
        i = bc_idx[0]
        bc_idx[0] += 1
        t = pstile([128, n], f"bc_{tag}{i}")
        mm(t[:], (ones if ones is not None else c_oner)[:], row_ap,
           start=True, stop=True)
        s = spool.tile([128, n], F32, tag=f"bc_{tag}", name=f"bcs_{tag}{i}")
        dve.tensor_copy(s[:], t[:])
        return s

    # ---- rms scale for a [128, F] column-layout vector, D = 128*F ----
    rm_idx = [0]

    def global_rms_scale(v, F, D, tag):
        i = rm_idx[0]
        rm_idx[0] += 1
        sq = spool.tile([128, F], F32, tag=f"rm_sq_{tag}", name=f"rm_sq_{tag}{i}")
        dve.tensor_mul(sq[:], v[:], v[:])
        cs = pstile([1, F], f"rm_cs_{tag}{i}")
        mm(cs[:], c_one[:], sq[:], start=True, stop=True)
        tot = spool.tile([1, 1], F32, tag=f"rm_t_{tag}", name=f"rm_t_{tag}{i}")
        dve.reduce_sum(tot[:], cs[:], axis=AX.X)
        tt = spool.tile([1, 1], F32, tag=f"rm_tt_{tag}", name=f"rm_tt_{tag}{i}")
        dve.tensor_scalar(tt[:], tot[:], 1.0 / D, EPS,
                          op0=mybir.AluOpType.mult, op1=mybir.AluOpType.add)
        sr = spool.tile([1, 1], F32, tag=f"rm_s_{tag}", name=f"rm_s_{tag}{i}")
        act.activation(sr[:], tt[:], AF.Sqrt)
        rs = spool.tile([1, 1], F32, tag=f"rm_r_{tag}", name=f"rm_r_{tag}{i}")
        dve.reciprocal(rs[:], sr[:])
        scp = pstile([128, 1], f"rm_b_{tag}{i}")
        mm(scp[:], c_oner[:], rs[:], start=True, stop=True)
        scs = spool.tile([128, 1], F32, tag=f"rm_bs_{tag}", name=f"rm_bs_{tag}{i}")
        dve.tensor_copy(scs[:], scp[:])
        return scs

    # =========================== PLE stage ===============================
    xple = state.tile([128, 1], F32, tag="xple", name="xple")
    tmp8 = state.tile([128, 8], F32, tag="tmp8", name="tmp8")
    dve.tensor_mul(tmp8[:], c_hid[:], c_xm[:])
    dve.reduce_sum(xple[:], tmp8[:], axis=AX.X)

    pj = pstile([128, 60], "pj")
    for m in range(60):
        mmr(pj[:, m : m + 1], c_plew[:, 128 * m : 128 * (m + 1)], xple[:],
            start=True, stop=True)
    # PLE partials ride along with layer-0's qkv AR (see loop below).
    ple_pending = [pj]

    def ple_finish(projs):
        # group-rms over 30 groups of 256 (cols 2g, 2g+1)
        psq = state.tile([128, 60], F32, tag="psq", name="psq")
        dve.tensor_mul(psq[:], projs[:], projs[:])
        pcs = pstile([1, 60], "pcs")
        mm(pcs[:], c_one[:], psq[:], start=True, stop=True)
        pcss = state.tile([1, 60], F32, tag="pcss", name="pcss")
        dve.tensor_copy(pcss[:], pcs[:])
        g2 = state.tile([1, 30], F32, tag="g2", name="g2")
        dve.tensor_add(g2[:], pcss[:, 0::2], pcss[:, 1::2])
        g2e = state.tile([1, 30], F32, tag="g2e", name="g2e")
        dve.tensor_scalar(g2e[:], g2[:], 1.0 / PLE, EPS,
                          op0=mybir.AluOpType.mult, op1=mybir.AluOpType.add)
        gsr = state.tile([1, 30], F32, tag="gsr", name="gsr")
        act.activation(gsr[:], g2e[:], AF.Sqrt)
        grs = state.tile([1, 30], F32, tag="grs", name="grs")
        dve.reciprocal(grs[:], gsr[:])
        s60 = state.tile([1, 60], F32, tag="s60", name="s60")
        dve.tensor_copy(s60[:, 0::2], grs[:])
        dve.tensor_copy(s60[:, 1::2], grs[:])
        sc60 = bcast_cols(s60[:], 60, "p60")
        normed = state.tile([128, 60], F32, tag="normed", name="normed")
        dve.tensor_mul(normed[:], projs[:], sc60[:])
        # plenw and raw are pre-scaled by 2^-0.5 on host
        plec = state.tile([128, 60], F32, tag="plec", name="plec")
        dve.tensor_mul(plec[:], normed[:], c_plenw[:])
        dve.tensor_add(plec[:], plec[:], c_raw[:])
        nc.sync.dma_start(out=ple_out[:], in_=plec[:])
        return plec

    plec = None

    # =========================== layers ==================================
    h = state.tile([128, 8], F32, tag="h", name="h0")
    dve.tensor_copy(h[:], c_hid[:])

    for l in range(NL):
        w_qkv = wpool.tile([128, 2048], F32, tag="w_qkv", name=f"w_qkv{l}")
        nc.sync.dma_start(out=w_qkv[:], in_=wqkv[l])
        w_o = wpool.tile([128, 1024], F32, tag="w_o", name=f"w_o{l}")
        nc.sync.dma_start(out=w_o[:], in_=wo[l])
        w_gu = wpool.tile([128, 4096], F32, tag="w_gu", name=f"w_gu{l}")
        nc.sync.dma_start(out=w_gu[:], in_=wgu[l])
        w_d = wpool.tile([128, 2048], F32, tag="w_d", name=f"w_d{l}")
        nc.sync.dma_start(out=w_d[:], in_=wd[l])
        w_pl = wpool.tile([128, 256], F32, tag="w_pl", name=f"w_pl{l}")
        nc.sync.dma_start(out=w_pl[:], in_=wpl[l])
        t_kt = wpool.tile([128, 2 * NP_], F32, tag="t_kt", name=f"t_kt{l}")
        nc.sync.dma_start(out=t_kt[:], in_=kt[l])
        vsb = wpool.tile([NP_, 128], F32, tag="vsb", name=f"vsb{l}")
        nc.sync.dma_start(out=vsb[:], in_=vv[l])

        # ---- x = rms(h) * ln1 ----
        hsc = global_rms_scale(h, 8, H, "h1")
        x = spool.tile([128, 8], F32, tag="x", name=f"x{l}")
        dve.tensor_scalar_mul(x[:], h[:], hsc[:])

        # ---- qkv partials ----
        xc8 = spool.tile([128, 8], F32, tag="xc8", name=f"xc8{l}")
        dve.tensor_mul(xc8[:], x[:], c_xm[:])
        xcs = spool.tile([128, 1], F32, tag="xcs", name=f"xcs{l}")
        dve.reduce_sum(xcs[:], xc8[:], axis=AX.X)
        pqkv = pstile([128, 16], f"pqkv{l}")
        for m in range(16):
            mmr(pqkv[:, m : m + 1], w_qkv[:, 128 * m : 128 * (m + 1)], xcs[:],
                start=True, stop=True)
        if l == 0:
            sqkv = spool.tile([128, 76], F32, tag="sqkv0", name="sqkv0")
            dve.tensor_copy(sqkv[:, 0:16], pqkv[:])
            dve.tensor_copy(sqkv[:, 16:76], ple_pending[0][:])
            comb = allreduce(sqkv[:], 76, "ar1w")
            qkvc = comb[:, 0:16]
            plec = ple_finish(comb[:, 16:76])
        else:
            sqkv = spool.tile([128, 16], F32, tag="sqkv", name=f"sqkv{l}")
            dve.tensor_copy(sqkv[:], pqkv[:])
            qkvc = allreduce(sqkv[:], 16, "ar1")
        q = qkvc[:, 0:8]
        k = qkvc[:, 8:12]
        v = qkvc[:, 12:16]

        # v straight to cache output
        nc.sync.dma_start(out=v_out[l], in_=v)

        # ---- q rms (with 1/16 folded) + rope ----
        qksq = spool.tile([128, 12], F32, tag="qksq", name=f"qksq{l}")
        dve.tensor_mul(qksq[:], qkvc[:, 0:12], qkvc[:, 0:12])
        qkcs = pstile([1, 12], f"qkcs{l}")
        mm(qkcs[:], c_one[:], qksq[:], start=True, stop=True)
        qkss = spool.tile([1, 12], F32, tag="qkss", name=f"qkss{l}")
        dve.tensor_copy(qkss[:], qkcs[:])
        qg = spool.tile([1, 4], F32, tag="qg", name=f"qg{l}")
        dve.tensor_add(qg[:], qkss[:, 0:8:2], qkss[:, 1:8:2])
        qge = spool.tile([1, 4], F32, tag="qge", name=f"qge{l}")
        dve.tensor_scalar(qge[:], qg[:], 256.0 / HD, 256.0 * EPS,
                          op0=mybir.AluOpType.mult, op1=mybir.AluOpType.add)
        qsr = spool.tile([1, 4], F32, tag="qsr", name=f"qsr{l}")
        act.activation(qsr[:], qge[:], AF.Sqrt)
        qrs = spool.tile([1, 4], F32, tag="qrs", name=f"qrs{l}")
        dve.reciprocal(qrs[:], qsr[:])
        qs8 = spool.tile([1, 8], F32, tag="qs8", name=f"qs8{l}")
        dve.tensor_copy(qs8[:, 0::2], qrs[:])
        dve.tensor_copy(qs8[:, 1::2], qrs[:])
        qsc = bcast_cols(qs8[:], 8, "q")  # rsqrt/16 folded via 256x sqrt arg
        qn_ = spool.tile([128, 8], F32, tag="qn_", name=f"qn_{l}")
        dve.tensor_mul(qn_[:], q, qsc[:])
        rotq = spool.tile([128, 8], F32, tag="rotq", name=f"rotq{l}")
        dve.tensor_scalar_mul(rotq[:, 0::2], qn_[:, 1::2], -1.0)
        dve.tensor_copy(rotq[:, 1::2], qn_[:, 0::2])
        qr = spool.tile([128, 8], F32, tag="qr", name=f"qr{l}")
        dve.tensor_mul(qr[:], qn_[:], c_cosq[:, 8 * l : 8 * l + 8])
        dve.tensor_mul(rotq[:], rotq[:], c_sinq[:, 8 * l : 8 * l + 8])
        dve.tensor_add(qr[:], qr[:], rotq[:])

        # ---- k rms + rope ----
        kg = spool.tile([1, 2], F32, tag="kg", name=f"kg{l}")
        dve.tensor_add(kg[:], qkss[:, 8:12:2], qkss[:, 9:12:2])
        kge = spool.tile([1, 2], F32, tag="kge", name=f"kge{l}")
        dve.tensor_scalar(kge[:], kg[:], 1.0 / HD, EPS,
                          op0=mybir.AluOpType.mult, op1=mybir.AluOpType.add)
        ksr = spool.tile([1, 2], F32, tag="ksr", name=f"ksr{l}")
        act.activation(ksr[:], kge[:], AF.Sqrt)
        krs = spool.tile([1, 2], F32, tag="krs", name=f"krs{l}")
        dve.reciprocal(krs[:], ksr[:])
        ks4 = spool.tile([1, 4], F32, tag="ks4", name=f"ks4{l}")
        dve.tensor_copy(ks4[:, 0::2], krs[:])
        dve.tensor_copy(ks4[:, 1::2], krs[:])
        ksc = bcast_cols(ks4[:], 4, "k")
        dwe = spool.tile([1, 1], F32, tag="dwe", name=f"dwe{l}")
        act.activation(dwe[:], krs[:, 0:1], AF.Exp)  # pre-warm Exp table
        kn_ = spool.tile([128, 4], F32, tag="kn_", name=f"kn_{l}")
        dve.tensor_mul(kn_[:], k, ksc[:])
        rotk = spool.tile([128, 4], F32, tag="rotk", name=f"rotk{l}")
        dve.tensor_scalar_mul(rotk[:, 0::2], kn_[:, 1::2], -1.0)
        dve.tensor_copy(rotk[:, 1::2], kn_[:, 0::2])
        kr = spool.tile([128, 4], F32, tag="kr", name=f"kr{l}")
        dve.tensor_mul(kr[:], kn_[:], c_cosk[:, 4 * l : 4 * l + 4])
        dve.tensor_mul(rotk[:], rotk[:], c_sink[:, 4 * l : 4 * l + 4])
        dve.tensor_add(kr[:], kr[:], rotk[:])
        nc.sync.dma_start(out=k_out[l], in_=kr[:])

        # ---- my q head halves ----
        tq = spool.tile([128, 8], F32, tag="tq", name=f"tq{l}")
        dve.tensor_mul(tq[:], qr[:], c_qma[:])
        qa = spool.tile([128, 1], F32, tag="qa", name=f"qa{l}")
        dve.reduce_sum(qa[:], tq[:], axis=AX.X)
        dve.tensor_mul(tq[:], qr[:], c_qmb[:])
        qb = spool.tile([128, 1], F32, tag="qb", name=f"qb{l}")
        dve.reduce_sum(qb[:], tq[:], axis=AX.X)

        # ---- scores over cache rows 0..99 ----
        psc = pstile([1, POS + 1], f"psc{l}")
        mm(psc[:, 0:NP_], qa[:], t_kt[:, 0:NP_], start=True, stop=False)
        mm(psc[:, 0:NP_], qb[:], t_kt[:, NP_ : 2 * NP_], start=False, stop=True)

        # ---- score at row 100 (new k) ----
        krep = spool.tile([128, 8], F32, tag="krep", name=f"krep{l}")
        kr_b = (kr.rearrange("p (u e) -> p u e", e=2)
                .broadcast_to((128, 2, 2, 2))
                .rearrange("p u e r -> p u r e"))
        dve.tensor_mul(
            krep.rearrange("p (u r e) -> p u r e", u=2, r=2, e=2),
            qr.rearrange("p (u r e) -> p u r e", u=2, r=2, e=2),
            kr_b)
        dve.tensor_mul(krep[:], krep[:], c_qhm[:])
        s1cs = pstile([1, 8], f"s1cs{l}")
        mm(s1cs[:], c_one[:], krep[:], start=True, stop=True)
        s100 = spool.tile([1, 1], F32, tag="s100", name=f"s100{l}")
        dve.reduce_sum(s100[:], s1cs[:], axis=AX.X)

        sc = spool.tile([1, POS + 1], F32, tag="sc", name=f"sc{l}")
        dve.tensor_copy(sc[:, 0:NP_], psc[:, 0:NP_])
        dve.tensor_copy(sc[:, POS : POS + 1], s100[:])

        # ---- softmax ----
        mx = spool.tile([1, 1], F32, tag="mx", name=f"mx{l}")
        dve.reduce_max(mx[:], sc[:], axis=AX.X)
        nmx = spool.tile([1, 1], F32, tag="nmx", name=f"nmx{l}")
        dve.tensor_scalar_mul(nmx[:], mx[:], -1.0)
        e = spool.tile([1, POS + 1], F32, tag="e", name=f"e{l}")
        esum = spool.tile([1, 1], F32, tag="esum", name=f"esum{l}")
        act.activation(e[:], sc[:], AF.Exp, bias=nmx[:], scale=1.0,
                       accum_out=esum[:])
        dws = spool.tile([1, 1], F32, tag="dws", name=f"dws{l}")
        act.activation(dws[:], esum[:], AF.Sqrt)  # pre-warm Sqrt table
        rcp = spool.tile([1, 1], F32, tag="rcp", name=f"rcp{l}")
        dve.reciprocal(rcp[:], esum[:])
        p = spool.tile([1, POS + 1], F32, tag="p", name=f"p{l}")
        dve.tensor_scalar_mul(p[:], e[:], rcp[:])

        # ---- p[:100] to column; v_new row ----
        pct = pstile([NP_, 1], f"pct{l}")
        nc.tensor.transpose(pct[:], p[:, 0:NP_], c_id[0:1, 0:1])
        pcol = spool.tile([NP_, 1], F32, tag="pcol", name=f"pcol{l}")
        dve.tensor_copy(pcol[:], pct[:])

        tv8 = spool.tile([128, 4], F32, tag="tv8", name=f"tv8{l}")
        dve.tensor_mul(tv8[:], v, c_vm[:])
        vsl = spool.tile([128, 1], F32, tag="vsl", name=f"vsl{l}")
        dve.reduce_sum(vsl[:], tv8[:], axis=AX.X)
        vrt = pstile([1, 128], f"vrt{l}")
        nc.tensor.transpose(vrt[:], vsl[:], c_id[:])
        vrow = spool.tile([1, 128], F32, tag="vrow", name=f"vrow{l}")
        dve.tensor_copy(vrow[:], vrt[:])

        # ---- att = V[0:100]^T p[0:100] + p[100] * v_new ----
        patt = pstile([128, 1], f"patt{l}")
        mm(patt[:], vsb[:], pcol[:], start=True, stop=False)
        mm(patt[:], vrow[:], p[:, POS : POS + 1], start=False, stop=True)
        attc = spool.tile([128, 1], F32, tag="attc", name=f"attc{l}")
        dve.tensor_copy(attc[:], patt[:])

        # ---- o partials ----
        po = pstile([128, 8], f"po{l}")
        for m in range(8):
            mmr(po[:, m : m + 1], w_o[:, 128 * m : 128 * (m + 1)], attc[:],
                start=True, stop=True)
        so = spool.tile([128, 8], F32, tag="so", name=f"so{l}")
        dve.tensor_copy(so[:], po[:])
        oc = allreduce(so[:], 8, "ar2")
        h2 = state.tile([128, 8], F32, tag="h2", name=f"h2_{l}")
        dve.tensor_add(h2[:], h[:], oc[:])

        # ---- x2 = rms(h2) * ln2 ----
        h2sc = global_rms_scale(h2, 8, H, "h2")
        dwg = spool.tile([1, 1], F32, tag="dwg", name=f"dwg{l}")
        act.activation(dwg[:], h2sc[0:1, :], AF.Gelu_apprx_tanh)  # pre-warm
        x2 = spool.tile([128, 8], F32, tag="x2", name=f"x2{l}")
        dve.tensor_scalar_mul(x2[:], h2[:], h2sc[:])

        # ---- mlp g/u (col shard: full x2 contraction) ----
        pg = pstile([128, 2], f"pg{l}")
        pu = pstile([128, 2], f"pu{l}")
        for m in range(2):
            for kk in range(8):
                mmr(pg[:, m : m + 1],
                    w_gu[:, 256 * kk + 128 * m : 256 * kk + 128 * m + 128],
                    x2[:, kk : kk + 1], start=(kk == 0), stop=(kk == 7))
        for m in range(2):
            for kk in range(8):
                mmr(pu[:, m : m + 1],
                    w_gu[:, 2048 + 256 * kk + 128 * m : 2048 + 256 * kk + 128 * m + 128],
                    x2[:, kk : kk + 1], start=(kk == 0), stop=(kk == 7))
        ga = spool.tile([128, 2], F32, tag="ga", name=f"ga{l}")
        act.activation(ga[:], pg[:], AF.Gelu_apprx_tanh)
        mc = spool.tile([128, 2], F32, tag="mc", name=f"mc{l}")
        dve.tensor_mul(mc[:], ga[:], pu[:])
        dwq = spool.tile([1, 1], F32, tag="dwq", name=f"dwq{l}")
        act.activation(dwq[:], ga[0:1, 0:1], AF.Sqrt)  # pre-warm Sqrt table

        # ---- d partials (row shard of Wd over my 256 dff dims) ----
        pd = pstile([128, 8], f"pd{l}")
        for m in range(8):
            for kk in range(2):
                mmr(pd[:, m : m + 1],
                    w_d[:, 1024 * kk + 128 * m : 1024 * kk + 128 * m + 128],
                    mc[:, kk : kk + 1], start=(kk == 0), stop=(kk == 1))
        # ---- ple contribution (col shard of Wpl; placed by col mask) ----
        ppl = pstile([128, 1], f"ppl{l}")
        for kk in range(2):
            mmr(ppl[:], w_pl[:, 128 * kk : 128 * kk + 128],
                plec[:, 2 * l + kk : 2 * l + kk + 1],
                start=(kk == 0), stop=(kk == 1))
        plp = spool.tile([128, 1], F32, tag="plp", name=f"plp{l}")
        dve.tensor_copy(plp[:], ppl[:])
        sd = spool.tile([128, 8], F32, tag="sd", name=f"sd{l}")
        dve.tensor_scalar_mul(sd[:], c_cm8[:], plp[:])
        dve.tensor_add(sd[:], sd[:], pd[:])

        dc = allreduce(sd[:], 8, "ar3")
        hn = state.tile([128, 8], F32, tag="h", name=f"h{l + 1}")
        dve.tensor_add(hn[:], h2[:], dc[:])
        h = hn

    nc.sync.dma_start(out=h_out[:], in_=h[:])

    stack.close()


# ---------------------------------------------------------------------------
# host-side shard prep
def _col(vec, parts=128):
    """[D] -> [128, D/128] column layout (col f = dims 128f..128f+128)."""
    v = np.asarray(vec, dtype=np.float32).reshape(-1)
    return np.ascontiguousarray(v.reshape(-1, parts).T)


def _uncol(mat):
    return np.ascontiguousarray(mat.T).reshape(-1)


def _host_prep(inp):
    f32 = np.float32
    ln1w = np.asarray(inp["ln1"], f32)[:, :, None]
    ln2w = np.asarray(inp["ln2"], f32)[:, :, None]
    Wq = np.asarray(inp["Wq"], f32) * ln1w
    Wk = np.asarray(inp["Wk"], f32) * ln1w
    Wv = np.asarray(inp["Wv"], f32) * ln1w
    Wo = np.asarray(inp["Wo"], f32)
    Wg = np.asarray(inp["Wg"], f32) * ln2w
    Wu = np.asarray(inp["Wu"], f32) * ln2w
    Wd = np.asarray(inp["Wd"], f32)
    Wpl = np.asarray(inp["Wpl"], f32)
    plw = np.asarray(inp["ple_proj_W"], f32)
    qn = np.asarray(inp["qn"], f32)
    kn = np.asarray(inp["kn"], f32)
    ln1 = np.asarray(inp["ln1"], f32)
    ln2 = np.asarray(inp["ln2"], f32)
    Ks = np.asarray(inp["K_sliding_in"], f32)
    Vs = np.asarray(inp["V_sliding_in"], f32)
    Kf = np.asarray(inp["K_full_in"], f32)
    Vf = np.asarray(inp["V_full_in"], f32)
    cos_s = np.asarray(inp["cos_s"], f32).reshape(HD)
    sin_s = np.asarray(inp["sin_s"], f32).reshape(HD)
    cos_f = np.asarray(inp["cos_f"], f32).reshape(HD)
    sin_f = np.asarray(inp["sin_f"], f32).reshape(HD)
    hid = np.asarray(inp["hidden_states"], f32).reshape(H)
    raw = np.asarray(inp["per_layer_raw"], f32).reshape(NTOT * PLE)
    plnw = np.asarray(inp["ple_norm_w"], f32).reshape(PLE)

    # per-layer cache K/V (sliding/full), rows 0..99 of the right kv slice
    kcache = []
    vcache = []
    si = fi = 0
    for l in range(NL):
        if l in FULL_LAYERS:
            kcache.append(Kf[fi, 0])
            vcache.append(Vf[fi, 0])
            fi += 1
        else:
            kcache.append(Ks[si, 0])
            vcache.append(Vs[si, 0])
            si += 1

    # per-layer cos/sin columns with qn/kn folded
    cosq_h = np.zeros((128, 120), f32)
    sinq_h = np.zeros((128, 120), f32)
    cosk_h = np.zeros((128, 60), f32)
    sink_h = np.zeros((128, 60), f32)
    ln1_h = np.zeros((128, 120), f32)
    ln2_h = np.zeros((128, 120), f32)
    for l in range(NL):
        cos = cos_f if l in FULL_LAYERS else cos_s
        sin = sin_f if l in FULL_LAYERS else sin_s
        qnl, knl = qn[l], kn[l]
        qn_sw = np.concatenate([qnl[128:], qnl[:128]])
        kn_sw = np.concatenate([knl[128:], knl[:128]])
        cq = _col(cos * qnl)      # [128,2]
        sq_ = _col(sin * qn_sw)
        ck = _col(cos * knl)
        sk = _col(sin * kn_sw)
        for u in range(NH):
            cosq_h[:, 8 * l + 2 * u : 8 * l + 2 * u + 2] = cq
            sinq_h[:, 8 * l + 2 * u : 8 * l + 2 * u + 2] = sq_
        for u in range(NKV):
            cosk_h[:, 4 * l + 2 * u : 4 * l + 2 * u + 2] = ck
            sink_h[:, 4 * l + 2 * u : 4 * l + 2 * u + 2] = sk
        ln1_h[:, 8 * l : 8 * l + 8] = _col(ln1[l])
        ln2_h[:, 8 * l : 8 * l + 8] = _col(ln2[l])

    common = dict(
        hidc=_col(hid),
        rawc=_col(raw) * np.float32(2 ** -0.5),
        plenwc=np.tile(_col(plnw), (1, 30)).astype(f32) * np.float32(2 ** -0.5),
        ln1c=ln1_h, ln2c=ln2_h,
        cosq=cosq_h, sinq=sinq_h, cosk=cosk_h, sink=sink_h,
        onescol=np.ones((128, 1), f32),
        onesrow=np.ones((1, 128), f32),
        onesrow16=np.full((1, 128), HD ** -0.5, f32),
        ident=np.eye(128, dtype=f32),
    )

    in_maps = []
    for c in range(N_CORES):
        r0 = 128 * c
        j = c // 2          # q head
        half = c % 2
        jk = c // 4         # kv head
        wqkv_c = np.concatenate(
            [Wq[:, r0 : r0 + 128, :], Wk[:, r0 : r0 + 128, :],
             Wv[:, r0 : r0 + 128, :]], axis=2)
        worow = 256 * j + 128 * half
        wo_c = Wo[:, worow : worow + 128, :]
        gcols = slice(256 * c, 256 * c + 256)
        wg_c = (Wg[:, :, gcols].reshape(NL, 8, 128, 256)
                .transpose(0, 2, 1, 3).reshape(NL, 128, 2048))
        wu_c = (Wu[:, :, gcols].reshape(NL, 8, 128, 256)
                .transpose(0, 2, 1, 3).reshape(NL, 128, 2048))
        wgu_c = np.concatenate([wg_c, wu_c], axis=2)
        wd_c = (Wd[:, 256 * c : 256 * c + 256, :].reshape(NL, 2, 128, 1024)
                .transpose(0, 2, 1, 3).reshape(NL, 128, 2048))
        wpl_c = (Wpl[:, :, 128 * c : 128 * c + 128].reshape(NL, 2, 128, 128)
                 .transpose(0, 2, 1, 3).reshape(NL, 128, 256))
        plew_c = plw[r0 : r0 + 128, :] * np.float32(H ** -0.5)

        kt_c = np.zeros((NL, 128, 2 * NP_), f32)
        vv_c = np.zeros((NL, NP_, 128), f32)
        for l in range(NL):
            kk = kcache[l][0:NP_, 256 * jk : 256 * jk + 256]  # [100,256]
            ktt = np.ascontiguousarray(kk.T).reshape(2, 128, NP_)
            kt_c[l, :, 0:NP_] = ktt[0]
            kt_c[l, :, NP_ : 2 * NP_] = ktt[1]
            vcol0 = 256 * jk + 128 * half
            vv_c[l] = vcache[l][0:NP_, vcol0 : vcol0 + 128]

        def onehot(ncols, hot):
            m = np.zeros((128, ncols), f32)
            for hh in (hot if isinstance(hot, (list, tuple)) else [hot]):
                m[:, hh] = 1.0
            return m

        im = dict(common)
        im.update(
            wqkv=np.ascontiguousarray(wqkv_c),
            wo=np.ascontiguousarray(wo_c),
            wgu=np.ascontiguousarray(wgu_c),
            wd=np.ascontiguousarray(wd_c),
            wpl=np.ascontiguousarray(wpl_c),
            plew=np.ascontiguousarray(plew_c),
            kt=kt_c,
            vv=vv_c,
            xmask=onehot(8, c),
            qma=onehot(8, 2 * j),
            qmb=onehot(8, 2 * j + 1),
            qhm=onehot(8, [2 * j, 2 * j + 1]),
            vmask=onehot(4, 2 * jk + half),
            cmask8=onehot(8, c),
        )
        in_maps.append(im)
    return in_maps


_CACHED_NC = None


def _get_nc():
    global _CACHED_NC
    if _CACHED_NC is None:
        _CACHED_NC = build_nc()
    return _CACHED_NC


def kernel(_trace=False, **inputs):
    nc = _get_nc()
    in_maps = _host_prep(inputs)
    res = run_bass_kernel_spmd(nc, in_maps, CORE_IDS, trace=_trace)
    r0 = res.results[0]

    h = _uncol(r0["h_out"]).reshape(1, 1, H)
    ple = _uncol(r0["ple_out"]).reshape(1, 1, NTOT * PLE)
    k_new = np.stack([_uncol(r0["k_out"][l]) for l in range(NL)])  # [15,512]
    v_new = np.stack([_uncol(r0["v_out"][l]) for l in range(NL)])

    Ks = np.array(np.asarray(inputs["K_sliding_in"], np.float32), copy=True)
    Vs = np.array(np.asarray(inputs["V_sliding_in"], np.float32), copy=True)
    Kf = np.array(np.asarray(inputs["K_full_in"], np.float32), copy=True)
    Vf = np.array(np.asarray(inputs["V_full_in"], np.float32), copy=True)
    si = fi = 0
    kv13 = kv14 = None
    for l in range(NL):
        if l in FULL_LAYERS:
            Kf[fi, 0, POS, :] = k_new[l]
            Vf[fi, 0, POS, :] = v_new[l]
            fi += 1
        else:
            Ks[si, 0, POS, :] = k_new[l]
            Vs[si, 0, POS, :] = v_new[l]
            si += 1
    kv13_k = k_new[13][:HD].reshape(1, 1, 1, HD)
    kv13_v = v_new[13][:HD].reshape(1, 1, 1, HD)
    kv14_k = k_new[14].reshape(1, 1, 1, NKV * HD)
    kv14_v = v_new[14].reshape(1, 1, 1, NKV * HD)

    out = (h, Ks, Vs, Kf, Vf, kv13_k, kv13_v, kv14_k, kv14_v, ple)
    if _trace:
        return out, res
    return out
